# revision 5
# baseline (speedup 1.0000x reference)
"""AdaptiveRouter (MoE routing) Trainium2 kernel — 8-core data-parallel.

Strategy:
  - Shard token dim N=32768 across 8 cores (4096 tokens each).
  - Host-side input prep (layout only):
      * each core's x shard is transposed to [D=1024, 4096], split into bf16
        hi/lo halves, and stacked into one [2048, 4096] tensor so the gate
        matmul runs as 3 bf16 matmuls (x_hi@W_hi + x_hi@W_lo + x_lo@W_hi)
        with near-fp32 accuracy (measured ~5e-6 abs err) at bf16 TensorE rate.
      * all small TTHA weights are packed into 3 blob tensors (one
        partition-major [128,*] f32 blob, one [3,*] blob, one [1,*] blob)
        so weight loading is 3 DMA issues instead of ~22.
  - x-slab DMAs are issued FIRST on the sync engine (full 8-group prefetch,
    bufs=8) so the TensorE pipeline starts immediately; weight/TTHA DMAs go
    through gpsimd (SWDGE) to stay off the sync HWDGE FIFO.
  - The tiny batch-1 TTHA adapter is computed on every core (replicated,
    fp32, overlaps the x DMA stream). Activation-table switching minimized:
    Ln/Exp set for layernorm-rsqrt/softmax/softplus, Gelu set (which also
    holds tanh) for the rest; Identity (present in every set) is used for
    the fused bias-add epilogue on the Scalar engine.
  - Per 512-token group: 24 bf16 matmuls -> PSUM [64experts, 512tok],
    ACT Identity adds (b_gate + routing_bias) per partition on the psum->
    sbuf copy, PE-transpose to [128tok, 64], DVE Max8/MaxIndex top-2,
    batched top-2 softmax epilogue.
  - No collectives; host gathers per-core outputs.
"""

import sys

sys.path.insert(0, "/opt/trn_rl_repo")

import numpy as np
import ml_dtypes

import concourse.bass as bass
import concourse.mybir as mybir
import concourse.tile as tile
from concourse import bacc
from concourse.bass_utils import run_bass_kernel_spmd
from concourse.masks import make_identity

F32 = mybir.dt.float32
BF16 = mybir.dt.bfloat16
I32 = mybir.dt.int32
U32 = mybir.dt.uint32
AF = mybir.ActivationFunctionType
OP = mybir.AluOpType
AX = mybir.AxisListType

N_CORES = 8
N, D, E, K = 32768, 1024, 64, 2
NT = N // N_CORES            # 4096 tokens per core
H = 256
G_TOK = 512                  # tokens per matmul group
N_GROUPS = NT // G_TOK       # 8
TILES = NT // 128            # 32 token tiles of 128
DC = D // 128                # 8 contraction chunks

BF = ml_dtypes.bfloat16

# ---- wblob layout (partition-major [128, WCOLS] f32) ----
# catT_cost: [128, 3, 1]   cost features transposed (384 padded into 3 chunks)
# cat_hw:    [8, 1] in a 1-col region (rows 0:8)
# wc:        [128, 3, 256] (Wc [384,256] chunked)
# wh:        [8, 256] region (rows 0:8)
# wqkv:      [128, 2, 768]
# wo:        [128, 2, 256]
# wf:        [128, 2, 256]
# wo1:       [128, 2, 128]
# wo2:       [128, 64]
# wu1:       [128, 2, 64]
# wu2:       [64, 64] region (rows 0:64)
_W_SEGS = [
    ("catT", 3), ("cat_hw", 1), ("wc", 3 * 256), ("wh", 256),
    ("wqkv", 2 * 768), ("wo", 2 * 256), ("wf", 2 * 256),
    ("wo1", 2 * 128), ("wo2", 64), ("wu1", 2 * 64), ("wu2", 64),
]
_W_OFF = {}
_off = 0
for _nm, _w in _W_SEGS:
    _W_OFF[_nm] = _off
    _off += _w
WCOLS = _off

_B1_SEGS = [
    ("embb", 512), ("ln1g", 512), ("ln1b", 512), ("bf", 256), ("gf", 256),
    ("bef", 256), ("bo1", 128), ("bo2", 64), ("bu1", 64), ("bu2", 64),
    ("bgate", 64),
]
_B1_OFF = {}
_off = 0
for _nm, _w in _B1_SEGS:
    _B1_OFF[_nm] = _off
    _off += _w
B1COLS = _off


def _act(nc, out, in_, func, scale=1.0, bias=0.0):
    nc.scalar.activation(out, in_, func, scale=scale, bias=bias)


def _layer_norm_gelu(nc, sb, x_ap, g_ap, b_ap, out_ap, groups, width, tag):
    """out = gelu(LN(x) * g + b) on one partition; LN per group of `width`."""
    x3 = x_ap.rearrange("o (g w) -> o g w", g=groups)
    g3 = g_ap.rearrange("o (g w) -> o g w", g=groups)
    b3 = b_ap.rearrange("o (g w) -> o g w", g=groups)
    o3 = out_ap.rearrange("o (g w) -> o g w", g=groups)

    mu = sb.tile([1, groups], F32, tag=f"{tag}_mu")
    nc.vector.tensor_reduce(mu[:], x3, axis=AX.X, op=OP.add)
    nc.vector.tensor_scalar(mu[:], mu[:], 1.0 / width, None, op0=OP.mult)

    xc = sb.tile([1, groups, width], F32, tag=f"{tag}_xc")
    nc.vector.tensor_tensor(xc[:], x3, mu[:, :, None].to_broadcast([1, groups, width]),
                            op=OP.subtract)
    sq = sb.tile([1, groups, width], F32, tag=f"{tag}_sq")
    nc.vector.tensor_tensor(sq[:], xc[:], xc[:], op=OP.mult)
    var = sb.tile([1, groups], F32, tag=f"{tag}_var")
    nc.vector.tensor_reduce(var[:], sq[:], axis=AX.X, op=OP.add)
    nc.vector.tensor_scalar(var[:], var[:], 1.0 / width, 1e-5, op0=OP.mult, op1=OP.add)
    # inv_std = exp(-0.5 * ln(var+eps))   (stays in the natural_log_exp set)
    _act(nc, var[:], var[:], AF.Ln)
    _act(nc, var[:], var[:], AF.Exp, scale=-0.5)

    nc.vector.tensor_tensor(xc[:], xc[:], var[:, :, None].to_broadcast([1, groups, width]),
                            op=OP.mult)
    nc.vector.tensor_tensor(xc[:], xc[:], g3, op=OP.mult)
    nc.vector.tensor_tensor(xc[:], xc[:], b3, op=OP.add)
    _act(nc, o3, xc[:], AF.Gelu)


def build():
    nc = bacc.Bacc(target_bir_lowering=False)

    xz = nc.dram_tensor("xz", [2 * D, NT], BF16, kind="ExternalInput")
    wgz = nc.dram_tensor("wgz", [2 * D, E], BF16, kind="ExternalInput")
    wblob = nc.dram_tensor("wblob", [128, WCOLS], F32, kind="ExternalInput")
    bblob1 = nc.dram_tensor("bblob1", [1, B1COLS], F32, kind="ExternalInput")
    bblob3 = nc.dram_tensor("bblob3", [3, 1024], F32, kind="ExternalInput")

    out_w = nc.dram_tensor("out_w", [128, TILES * 2], F32, kind="ExternalOutput")
    out_i = nc.dram_tensor("out_i", [128, TILES * 2], I32, kind="ExternalOutput")
    out_rb = nc.dram_tensor("out_rb", [1, E], F32, kind="ExternalOutput")
    out_un = nc.dram_tensor("out_un", [1, E], F32, kind="ExternalOutput")

    with tile.TileContext(nc) as tc:
        with tc.tile_pool(name="const", bufs=1) as cs, \
             tc.tile_pool(name="tt", bufs=1) as ts, \
             tc.tile_pool(name="xs", bufs=8) as xs, \
             tc.tile_pool(name="wk", bufs=3) as wk, \
             tc.tile_pool(name="pmain", bufs=3, space="PSUM") as pmain, \
             tc.tile_pool(name="ptr", bufs=2, space="PSUM") as ptr, \
             tc.tile_pool(name="ptt", bufs=3, space="PSUM") as ptt:

            # ---- x slab prefetch: all 8 groups, sync-engine HWDGE ----
            slabs = []
            for g in range(N_GROUPS):
                xg = xs.tile([128, 2 * DC, G_TOK], BF16, tag="xz")
                nc.sync.dma_start(
                    xg[:], xz[:, g * G_TOK:(g + 1) * G_TOK].rearrange(
                        "(c p) t -> p c t", p=128))
                slabs.append(xg)
            wz_s = cs.tile([128, 2 * DC, E], BF16)
            nc.sync.dma_start(wz_s[:], wgz[:].rearrange("(c p) e -> p c e", p=128))

            # ---- weight/bias blobs via gpsimd (SWDGE) ----
            wb = cs.tile([128, WCOLS], F32)
            nc.gpsimd.dma_start(wb[:], wblob[:])
            b1 = cs.tile([1, B1COLS], F32)
            nc.gpsimd.dma_start(b1[:], bblob1[:])
            b3 = cs.tile([3, 1024], F32)
            nc.gpsimd.dma_start(b3[:], bblob3[:])

            ident = cs.tile([128, 128], F32)
            make_identity(nc, ident[:])

            def wseg(nm, rows=128):
                off = _W_OFF[nm]
                w = dict(_W_SEGS)[nm]
                return wb[0:rows, off:off + w]

            def b1seg(nm, w):
                off = _B1_OFF[nm]
                return b1[:, off:off + w]

            # persistent result buffers
            vbuf8 = cs.tile([128, TILES * 8], F32)
            ibuf8 = cs.tile([128, TILES * 8], U32)
            wbuf = cs.tile([128, TILES, 2], F32)
            obuf = cs.tile([128, TILES, 2], I32)

            # =========================================================
            # TTHA adapter (fp32, replicated per core)
            # =========================================================
            catT = wseg("catT").rearrange("p (c o) -> p c o", c=3)
            wc = wseg("wc").rearrange("p (c n) -> p c n", c=3)
            ps_emb = ptt.tile([1, 2 * H], F32, tag="ptt")
            for c in range(3):
                nc.tensor.matmul(ps_emb[:, 0:H], catT[:, c, :], wc[:, c, :],
                                 start=(c == 0), stop=(c == 2))
            nc.tensor.matmul(ps_emb[:, H:2 * H], wseg("cat_hw", rows=8),
                             wseg("wh", rows=8), start=True, stop=True)
            e0 = ts.tile([1, 2 * H], F32)
            nc.vector.tensor_tensor(e0[:], ps_emb[:], b1seg("embb", 512), op=OP.add)
            emb_act = ts.tile([1, 2 * H], F32)
            _layer_norm_gelu(nc, ts, e0[:], b1seg("ln1g", 512), b1seg("ln1b", 512),
                             emb_act[:], groups=2, width=H, tag="ln1")

            # combT [128, 2, 3]
            combT = ts.tile([128, 2, 3], F32)
            nc.vector.memset(combT[:], 0.0)
            for c in range(2):
                ps_cb = ptt.tile([128, 2], F32, tag="ptt")
                for j in range(2):
                    nc.tensor.transpose(ps_cb[:, j:j + 1],
                                        emb_act[0:1, (j * H + c * 128):(j * H + c * 128 + 128)],
                                        ident[0:1, 0:1])
                nc.vector.tensor_copy(combT[:, c, 0:2], ps_cb[:])

            # qkv = comb @ Wqkv + bqkv -> [3, 768]
            wqkv = wseg("wqkv").rearrange("p (c n) -> p c n", c=2)
            qkv_s = ts.tile([3, 3 * H], F32)
            for half in range(2):
                ps_q = ptt.tile([3, 384], F32, tag="ptt")
                for c in range(2):
                    nc.tensor.matmul(ps_q[:], combT[:, c, :],
                                     wqkv[:, c, (half * 384):(half * 384 + 384)],
                                     start=(c == 0), stop=(c == 1))
                nc.vector.tensor_tensor(qkv_s[:, half * 384:(half + 1) * 384], ps_q[:],
                                        b3[:, half * 384:(half + 1) * 384], op=OP.add)

            # qT (scaled) / kT packed 3 heads per chunk
            qT = ts.tile([128, 3, 3], F32)
            kT = ts.tile([128, 3, 3], F32)
            scale = 1.0 / np.sqrt(32.0)
            for c in range(3):
                w = 96 if c < 2 else 64
                ps_t = ptt.tile([128, 3], F32, tag="ptt")
                nc.tensor.transpose(ps_t[:w], qkv_s[0:3, c * 96:(c * 96 + w)],
                                    ident[0:3, 0:3])
                nc.vector.tensor_scalar(qT[:w, c, :], ps_t[:w], scale, None, op0=OP.mult)
                ps_t2 = ptt.tile([128, 3], F32, tag="ptt")
                nc.tensor.transpose(ps_t2[:w], qkv_s[0:3, (H + c * 96):(H + c * 96 + w)],
                                    ident[0:3, 0:3])
                nc.vector.tensor_copy(kT[:w, c, :], ps_t2[:w])

            ps_sc = ptt.tile([3, 24], F32, tag="ptt")
            for h in range(8):
                c, hh = divmod(h, 3)
                nc.tensor.matmul(ps_sc[:, h * 3:(h + 1) * 3],
                                 qT[hh * 32:(hh + 1) * 32, c, :],
                                 kT[hh * 32:(hh + 1) * 32, c, :],
                                 start=True, stop=True)
            s_sc = ts.tile([3, 8, 3], F32)
            nc.vector.tensor_copy(s_sc[:], ps_sc[:].rearrange("p (h j) -> p h j", h=8))
            rmax = ts.tile([3, 8], F32)
            nc.vector.tensor_reduce(rmax[:], s_sc[:], axis=AX.X, op=OP.max)
            nc.vector.tensor_tensor(s_sc[:], s_sc[:],
                                    rmax[:, :, None].to_broadcast([3, 8, 3]), op=OP.subtract)
            _act(nc, s_sc[:], s_sc[:], AF.Exp)
            rsum = ts.tile([3, 8], F32)
            nc.vector.tensor_reduce(rsum[:], s_sc[:], axis=AX.X, op=OP.add)
            nc.vector.reciprocal(rsum[:], rsum[:])
            nc.vector.tensor_tensor(s_sc[:], s_sc[:],
                                    rsum[:, :, None].to_broadcast([3, 8, 3]), op=OP.mult)

            at_s = ts.tile([3, 8, 3], F32)
            ps_at = ptt.tile([3, 24], F32, tag="ptt")
            for h in range(8):
                nc.tensor.transpose(ps_at[:, h * 3:(h + 1) * 3], s_sc[:, h, :],
                                    ident[0:3, 0:3])
            nc.vector.tensor_copy(at_s[:], ps_at[:].rearrange("p (h j) -> p h j", h=8))
            ps_ctx = ptt.tile([3, H], F32, tag="ptt")
            for h in range(8):
                nc.tensor.matmul(ps_ctx[:, h * 32:(h + 1) * 32], at_s[:, h, :],
                                 qkv_s[0:3, (2 * H + h * 32):(2 * H + (h + 1) * 32)],
                                 start=True, stop=True)
            ctx_s = ts.tile([3, H], F32)
            nc.vector.tensor_copy(ctx_s[:], ps_ctx[:])

            ctxT = ts.tile([128, 2, 3], F32)
            for c in range(2):
                ps_ct = ptt.tile([128, 3], F32, tag="ptt")
                nc.tensor.transpose(ps_ct[:], ctx_s[0:3, c * 128:(c + 1) * 128],
                                    ident[0:3, 0:3])
                nc.vector.tensor_copy(ctxT[:, c, :], ps_ct[:])
            wo = wseg("wo").rearrange("p (c n) -> p c n", c=2)
            ps_att = ptt.tile([3, H], F32, tag="ptt")
            for c in range(2):
                nc.tensor.matmul(ps_att[:], ctxT[:, c, :], wo[:, c, :],
                                 start=(c == 0), stop=(c == 1))
            att_s = ts.tile([3, H], F32)
            nc.vector.tensor_tensor(att_s[:], ps_att[:], b3[:, 768:768 + H], op=OP.add)

            third = ts.tile([3, 1], F32)
            nc.vector.memset(third[:], 1.0 / 3.0)
            ps_mean = ptt.tile([1, H], F32, tag="ptt")
            nc.tensor.matmul(ps_mean[:], third[:], att_s[:], start=True, stop=True)
            mean_s = ts.tile([1, H], F32)
            nc.vector.tensor_copy(mean_s[:], ps_mean[:])

            meanT = ts.tile([128, 2, 1], F32)
            for c in range(2):
                ps_mt = ptt.tile([128, 1], F32, tag="ptt")
                nc.tensor.transpose(ps_mt[:], mean_s[0:1, c * 128:(c + 1) * 128],
                                    ident[0:1, 0:1])
                nc.vector.tensor_copy(meanT[:, c, :], ps_mt[:])
            wf = wseg("wf").rearrange("p (c n) -> p c n", c=2)
            ps_f = ptt.tile([1, H], F32, tag="ptt")
            for c in range(2):
                nc.tensor.matmul(ps_f[:], meanT[:, c, :], wf[:, c, :],
                                 start=(c == 0), stop=(c == 1))
            f0 = ts.tile([1, H], F32)
            nc.vector.tensor_tensor(f0[:], ps_f[:], b1seg("bf", H), op=OP.add)
            fused = ts.tile([1, H], F32)
            _layer_norm_gelu(nc, ts, f0[:], b1seg("gf", H), b1seg("bef", H), fused[:],
                             groups=1, width=H, tag="ln2")

            fusedT = ts.tile([128, 2, 1], F32)
            for c in range(2):
                ps_ft = ptt.tile([128, 1], F32, tag="ptt")
                nc.tensor.transpose(ps_ft[:], fused[0:1, c * 128:(c + 1) * 128],
                                    ident[0:1, 0:1])
                nc.vector.tensor_copy(fusedT[:, c, :], ps_ft[:])

            # heads: compute both first-layer matmuls, then batch the gelus
            wo1 = wseg("wo1").rearrange("p (c n) -> p c n", c=2)
            ps_h1 = ptt.tile([1, H // 2], F32, tag="ptt")
            for c in range(2):
                nc.tensor.matmul(ps_h1[:], fusedT[:, c, :], wo1[:, c, :],
                                 start=(c == 0), stop=(c == 1))
            wu1 = wseg("wu1").rearrange("p (c n) -> p c n", c=2)
            ps_u1 = ptt.tile([1, H // 4], F32, tag="ptt")
            for c in range(2):
                nc.tensor.matmul(ps_u1[:], fusedT[:, c, :], wu1[:, c, :],
                                 start=(c == 0), stop=(c == 1))
            h1 = ts.tile([1, H // 2], F32)
            nc.vector.tensor_tensor(h1[:], ps_h1[:], b1seg("bo1", H // 2), op=OP.add)
            u1 = ts.tile([1, H // 4], F32)
            nc.vector.tensor_tensor(u1[:], ps_u1[:], b1seg("bu1", H // 4), op=OP.add)
            _act(nc, h1[:], h1[:], AF.Gelu)
            _act(nc, u1[:], u1[:], AF.Gelu)

            h1T = ts.tile([128, 1], F32)
            ps_h1t = ptt.tile([128, 1], F32, tag="ptt")
            nc.tensor.transpose(ps_h1t[:], h1[0:1, :], ident[0:1, 0:1])
            nc.vector.tensor_copy(h1T[:], ps_h1t[:])
            ps_rb = ptt.tile([1, E], F32, tag="ptt")
            nc.tensor.matmul(ps_rb[:], h1T[:], wseg("wo2"), start=True, stop=True)
            rb = ts.tile([1, E], F32)
            nc.vector.tensor_tensor(rb[:], ps_rb[:], b1seg("bo2", E), op=OP.add)
            _act(nc, rb[:], rb[:], AF.Tanh)  # tanh lives in the gelu set
            nc.gpsimd.dma_start(out_rb[:], rb[:])

            u1T = ts.tile([64, 1], F32)
            ps_u1t = ptt.tile([64, 1], F32, tag="ptt")
            nc.tensor.transpose(ps_u1t[:], u1[0:1, :], ident[0:1, 0:1])
            nc.vector.tensor_copy(u1T[:], ps_u1t[:])
            ps_u2 = ptt.tile([1, E], F32, tag="ptt")
            nc.tensor.matmul(ps_u2[:], u1T[:], wseg("wu2", rows=64), start=True, stop=True)
            un = ts.tile([1, E], F32)
            nc.vector.tensor_tensor(un[:], ps_u2[:], b1seg("bu2", E), op=OP.add)
            _act(nc, un[:], un[:], AF.Exp)
            nc.vector.tensor_scalar(un[:], un[:], 1.0, None, op0=OP.add)
            _act(nc, un[:], un[:], AF.Ln)
            nc.gpsimd.dma_start(out_un[:], un[:])

            bt = ts.tile([1, E], F32)
            nc.vector.tensor_tensor(bt[:], rb[:], b1seg("bgate", E), op=OP.add)
            ps_bt = ptt.tile([64, 1], F32, tag="ptt")
            nc.tensor.transpose(ps_bt[:], bt[0:1, :], ident[0:1, 0:1])
            bias_T = ts.tile([64, 1], F32)
            nc.vector.tensor_copy(bias_T[:], ps_bt[:])

            # =========================================================
            # Main gate matmul + top-2
            # =========================================================
            for g in range(N_GROUPS):
                xg = slabs[g]
                ps_lg = pmain.tile([E, G_TOK], F32, tag="lg")
                n_mm = 3 * DC
                k = 0
                for c in range(DC):
                    # (whi, xhi), (whi, xlo), (wlo, xhi)
                    for (wc_i, xc_i) in ((c, c), (c, DC + c), (DC + c, c)):
                        nc.tensor.matmul(ps_lg[:], wz_s[:, wc_i, :], xg[:, xc_i, :],
                                         start=(k == 0), stop=(k == n_mm - 1))
                        k += 1

                # fused bias add on Scalar engine (Identity is in every set)
                adj = wk.tile([E, G_TOK], F32, tag="adj")
                nc.scalar.activation(adj[:], ps_lg[:], AF.Identity,
                                     bias=bias_T[:, 0:1], scale=1.0)

                for t in range(4):
                    tl = g * 4 + t
                    ps_tr = ptr.tile([128, E], F32, tag="tr")
                    nc.tensor.transpose(ps_tr[:], adj[:, t * 128:(t + 1) * 128],
                                        ident[0:E, 0:E])
                    adjT = wk.tile([128, E], F32, tag="adjT")
                    nc.vector.tensor_copy(adjT[:], ps_tr[:])
                    nc.vector.max(vbuf8[:, tl * 8:(tl + 1) * 8], adjT[:])
                    nc.vector.max_index(ibuf8[:, tl * 8:(tl + 1) * 8],
                                        vbuf8[:, tl * 8:(tl + 1) * 8], adjT[:])

            # ---- top-2 softmax epilogue ----
            v3 = vbuf8[:].rearrange("p (t k) -> p t k", k=8)
            i3 = ibuf8[:].rearrange("p (t k) -> p t k", k=8)
            d_t = wk.tile([128, TILES, 1], F32, tag="dt")
            nc.vector.tensor_tensor(d_t[:], v3[:, :, 1:2], v3[:, :, 0:1], op=OP.subtract)
            _act(nc, d_t[:], d_t[:], AF.Exp)
            s_t = wk.tile([128, TILES, 1], F32, tag="st")
            nc.vector.tensor_scalar(s_t[:], d_t[:], 1.0, None, op0=OP.add)
            nc.vector.reciprocal(wbuf[:, :, 0:1], s_t[:])
            nc.vector.tensor_tensor(wbuf[:, :, 1:2], d_t[:], wbuf[:, :, 0:1], op=OP.mult)
            nc.vector.tensor_copy(obuf[:, :, 0:1], i3[:, :, 0:1])
            nc.vector.tensor_copy(obuf[:, :, 1:2], i3[:, :, 1:2])
            nc.sync.dma_start(out_w[:], wbuf[:])
            nc.sync.dma_start(out_i[:], obuf[:])

    nc.finalize()
    return nc


_CACHE = {}


def _get_nc():
    if "nc" not in _CACHE:
        _CACHE["nc"] = build()
    return _CACHE["nc"]


def kernel(**inputs):
    f32 = np.float32
    g = {k: np.asarray(v, f32) for k, v in inputs.items()}
    x = g["x"]

    wg = g["W_gate"]
    wghi = wg.astype(BF)
    wglo = (wg - wghi.astype(f32)).astype(BF)
    wgz = np.concatenate([wghi, wglo], axis=0)

    wblob = np.zeros((128, WCOLS), f32)

    def put_w(nm, arr):
        off = _W_OFF[nm]
        arr = np.asarray(arr, f32)
        wblob[:arr.shape[0], off:off + arr.shape[1]] = arr

    cost = g["cost_features"][0]
    put_w("catT", np.ascontiguousarray(cost.reshape(3, 128).T))
    put_w("cat_hw", g["hardware_features"].reshape(8, 1))
    put_w("wc", g["Wc"].reshape(3, 128, 256).transpose(1, 0, 2).reshape(128, 3 * 256))
    put_w("wh", g["Wh"])
    put_w("wqkv", g["Wqkv"].reshape(2, 128, 768).transpose(1, 0, 2).reshape(128, 2 * 768))
    put_w("wo", g["Wo"].reshape(2, 128, 256).transpose(1, 0, 2).reshape(128, 2 * 256))
    put_w("wf", g["Wf"].reshape(2, 128, 256).transpose(1, 0, 2).reshape(128, 2 * 256))
    put_w("wo1", g["Wo1"].reshape(2, 128, 128).transpose(1, 0, 2).reshape(128, 2 * 128))
    put_w("wo2", g["Wo2"])
    put_w("wu1", g["Wu1"].reshape(2, 128, 64).transpose(1, 0, 2).reshape(128, 2 * 64))
    put_w("wu2", g["Wu2"])

    bblob1 = np.zeros((1, B1COLS), f32)

    def put_b(nm, arr):
        off = _B1_OFF[nm]
        arr = np.asarray(arr, f32).reshape(-1)
        bblob1[0, off:off + arr.size] = arr

    put_b("embb", np.concatenate([g["bc"], g["bh"]]))
    put_b("ln1g", np.concatenate([g["gc"], g["gh"]]))
    put_b("ln1b", np.concatenate([g["bec"], g["beh"]]))
    put_b("bf", g["bf"]); put_b("gf", g["gf"]); put_b("bef", g["bef"])
    put_b("bo1", g["bo1"]); put_b("bo2", g["bo2"])
    put_b("bu1", g["bu1"]); put_b("bu2", g["bu2"])
    put_b("bgate", g["b_gate"])

    bblob3 = np.zeros((3, 1024), f32)
    bblob3[:, 0:768] = g["bqkv"].reshape(1, 768)
    bblob3[:, 768:1024] = g["bo"].reshape(1, 256)

    shared = dict(wgz=wgz, wblob=wblob, bblob1=bblob1, bblob3=bblob3)

    in_maps = []
    for c in range(N_CORES):
        xs = np.ascontiguousarray(x[c * NT:(c + 1) * NT].T)
        xhi = xs.astype(BF)
        xlo = (xs - xhi.astype(f32)).astype(BF)
        in_maps.append(dict(shared, xz=np.concatenate([xhi, xlo], axis=0)))

    nc = _get_nc()
    res = run_bass_kernel_spmd(nc, in_maps, core_ids=list(range(N_CORES)))

    weights = np.empty((N, K), f32)
    top_idx = np.empty((N, K), np.int32)
    for c in range(N_CORES):
        r = res.results[c]
        weights[c * NT:(c + 1) * NT] = (
            r["out_w"].reshape(128, TILES, 2).transpose(1, 0, 2).reshape(NT, 2))
        top_idx[c * NT:(c + 1) * NT] = (
            r["out_i"].reshape(128, TILES, 2).transpose(1, 0, 2).reshape(NT, 2))
    rb = res.results[0]["out_rb"].reshape(1, E).astype(f32)
    un = res.results[0]["out_un"].reshape(1, E).astype(f32)
    return weights, top_idx, rb, un


# revision 7
# speedup vs baseline: 1.1440x; 1.1440x over previous
"""AdaptiveRouter (MoE routing) Trainium2 kernel — 8-core data-parallel.

Strategy:
  - Shard token dim N=32768 across 8 cores (4096 tokens each).
  - Host-side input prep (layout only):
      * each core's x shard is transposed to [D=1024, 4096], split into bf16
        hi/lo halves, and stacked into one [2048, 4096] tensor so the gate
        matmul runs as 3 bf16 matmuls (x_hi@W_hi + x_hi@W_lo + x_lo@W_hi)
        with near-fp32 accuracy (measured ~5e-6 abs err) at bf16 TensorE rate.
      * all small TTHA weights are packed into 3 blob tensors (one
        partition-major [128,*] f32 blob, one [3,*] blob, one [1,*] blob)
        so weight loading is 3 DMA issues instead of ~22.
  - x-slab DMAs are issued FIRST on the sync engine (full 8-group prefetch,
    bufs=8) so the TensorE pipeline starts immediately; weight/TTHA DMAs go
    through gpsimd (SWDGE) to stay off the sync HWDGE FIFO.
  - The tiny batch-1 TTHA adapter is computed on every core (replicated,
    fp32, overlaps the x DMA stream). Activation-table switching minimized:
    Ln/Exp set for layernorm-rsqrt/softmax/softplus, Gelu set (which also
    holds tanh) for the rest; Identity (present in every set) is used for
    the fused bias-add epilogue on the Scalar engine.
  - Per 512-token group: 24 bf16 matmuls -> PSUM [64experts, 512tok],
    ACT Identity adds (b_gate + routing_bias) per partition on the psum->
    sbuf copy, PE-transpose to [128tok, 64], DVE Max8/MaxIndex top-2,
    batched top-2 softmax epilogue.
  - No collectives; host gathers per-core outputs.
"""

import sys

sys.path.insert(0, "/opt/trn_rl_repo")

import numpy as np
import ml_dtypes

import concourse.bass as bass
import concourse.mybir as mybir
import concourse.tile as tile
from concourse import bacc
from concourse.bass_utils import run_bass_kernel_spmd
from concourse.masks import make_identity

F32 = mybir.dt.float32
BF16 = mybir.dt.bfloat16
I32 = mybir.dt.int32
U32 = mybir.dt.uint32
AF = mybir.ActivationFunctionType
OP = mybir.AluOpType
AX = mybir.AxisListType

N_CORES = 8
N, D, E, K = 32768, 1024, 64, 2
NT = N // N_CORES            # 4096 tokens per core
H = 256
G_TOK = 512                  # tokens per matmul group
N_GROUPS = NT // G_TOK       # 8
TILES = NT // 128            # 32 token tiles of 128
DC = D // 128                # 8 contraction chunks

BF = ml_dtypes.bfloat16

# ---- wblob layout (partition-major [128, WCOLS] f32) ----
# catT_cost: [128, 3, 1]   cost features transposed (384 padded into 3 chunks)
# cat_hw:    [8, 1] in a 1-col region (rows 0:8)
# wc:        [128, 3, 256] (Wc [384,256] chunked)
# wh:        [8, 256] region (rows 0:8)
# wqkv:      [128, 2, 768]
# wo:        [128, 2, 256]
# wf:        [128, 2, 256]
# wo1:       [128, 2, 128]
# wo2:       [128, 64]
# wu1:       [128, 2, 64]
# wu2:       [64, 64] region (rows 0:64)
_W_SEGS = [
    ("catT", 3), ("cat_hw", 1), ("wc", 3 * 256), ("wh", 256),
    ("wqkv", 2 * 768), ("wo", 2 * 256), ("wf", 2 * 256),
    ("wo1", 2 * 128), ("wo2", 64), ("wu1", 2 * 64), ("wu2", 64),
]
_W_OFF = {}
_off = 0
for _nm, _w in _W_SEGS:
    _W_OFF[_nm] = _off
    _off += _w
WCOLS = _off

_B1_SEGS = [
    ("embb", 512), ("ln1g", 512), ("ln1b", 512), ("bf", 256), ("gf", 256),
    ("bef", 256), ("bo1", 128), ("bo2", 64), ("bu1", 64), ("bu2", 64),
    ("bgate", 64),
]
_B1_OFF = {}
_off = 0
for _nm, _w in _B1_SEGS:
    _B1_OFF[_nm] = _off
    _off += _w
B1COLS = _off


def _act(nc, out, in_, func, scale=1.0, bias=0.0):
    nc.scalar.activation(out, in_, func, scale=scale, bias=bias)


def _layer_norm_gelu(nc, sb, x_ap, g_ap, b_ap, out_ap, groups, width, tag):
    """out = gelu(LN(x) * g + b) on one partition; LN per group of `width`."""
    x3 = x_ap.rearrange("o (g w) -> o g w", g=groups)
    g3 = g_ap.rearrange("o (g w) -> o g w", g=groups)
    b3 = b_ap.rearrange("o (g w) -> o g w", g=groups)
    o3 = out_ap.rearrange("o (g w) -> o g w", g=groups)

    mu = sb.tile([1, groups], F32, tag=f"{tag}_mu")
    nc.vector.tensor_reduce(mu[:], x3, axis=AX.X, op=OP.add)
    nc.vector.tensor_scalar(mu[:], mu[:], 1.0 / width, None, op0=OP.mult)

    xc = sb.tile([1, groups, width], F32, tag=f"{tag}_xc")
    nc.vector.tensor_tensor(xc[:], x3, mu[:, :, None].to_broadcast([1, groups, width]),
                            op=OP.subtract)
    sq = sb.tile([1, groups, width], F32, tag=f"{tag}_sq")
    nc.vector.tensor_tensor(sq[:], xc[:], xc[:], op=OP.mult)
    var = sb.tile([1, groups], F32, tag=f"{tag}_var")
    nc.vector.tensor_reduce(var[:], sq[:], axis=AX.X, op=OP.add)
    nc.vector.tensor_scalar(var[:], var[:], 1.0 / width, 1e-5, op0=OP.mult, op1=OP.add)
    # inv_std = exp(-0.5 * ln(var+eps))   (stays in the natural_log_exp set)
    _act(nc, var[:], var[:], AF.Ln)
    _act(nc, var[:], var[:], AF.Exp, scale=-0.5)

    nc.vector.tensor_tensor(xc[:], xc[:], var[:, :, None].to_broadcast([1, groups, width]),
                            op=OP.mult)
    nc.vector.tensor_tensor(xc[:], xc[:], g3, op=OP.mult)
    nc.vector.tensor_tensor(xc[:], xc[:], b3, op=OP.add)
    _act(nc, o3, xc[:], AF.Gelu)


def build():
    nc = bacc.Bacc(target_bir_lowering=False)

    xz = nc.dram_tensor("xz", [2 * D, NT], BF16, kind="ExternalInput")
    wgz = nc.dram_tensor("wgz", [2 * D, E], BF16, kind="ExternalInput")
    wblob = nc.dram_tensor("wblob", [128, WCOLS], F32, kind="ExternalInput")
    bblob1 = nc.dram_tensor("bblob1", [1, B1COLS], F32, kind="ExternalInput")
    bblob3 = nc.dram_tensor("bblob3", [3, 1024], F32, kind="ExternalInput")

    out_w = nc.dram_tensor("out_w", [128, TILES * 2], F32, kind="ExternalOutput")
    out_i = nc.dram_tensor("out_i", [128, TILES * 2], I32, kind="ExternalOutput")
    out_rb = nc.dram_tensor("out_rb", [1, E], F32, kind="ExternalOutput")
    out_un = nc.dram_tensor("out_un", [1, E], F32, kind="ExternalOutput")

    with tile.TileContext(nc) as tc:
        with tc.tile_pool(name="const", bufs=1) as cs, \
             tc.tile_pool(name="tt", bufs=1) as ts, \
             tc.tile_pool(name="xs", bufs=8) as xs, \
             tc.tile_pool(name="wk", bufs=3) as wk, \
             tc.tile_pool(name="pmain", bufs=5, space="PSUM") as pmain, \
             tc.tile_pool(name="ptr", bufs=1, space="PSUM") as ptr, \
             tc.tile_pool(name="ptt", bufs=2, space="PSUM") as ptt:

            # ---- gate weights via gpsimd so they beat the x-slab stream ----
            wz_s = cs.tile([128, 2 * DC, E], BF16)
            nc.gpsimd.dma_start(wz_s[:], wgz[:].rearrange("(c p) e -> p c e", p=128))

            # ---- x slab prefetch: all 8 groups, sync-engine HWDGE ----
            slabs = []
            for g in range(N_GROUPS):
                xg = xs.tile([128, 2 * DC, G_TOK], BF16, tag="xz")
                nc.sync.dma_start(
                    xg[:], xz[:, g * G_TOK:(g + 1) * G_TOK].rearrange(
                        "(c p) t -> p c t", p=128))
                slabs.append(xg)

            # ---- weight/bias blobs via gpsimd (SWDGE) ----
            wb = cs.tile([128, WCOLS], F32)
            nc.gpsimd.dma_start(wb[:], wblob[:])
            b1 = cs.tile([1, B1COLS], F32)
            nc.gpsimd.dma_start(b1[:], bblob1[:])
            b3 = cs.tile([3, 1024], F32)
            nc.gpsimd.dma_start(b3[:], bblob3[:])

            ident = cs.tile([128, 128], F32)
            make_identity(nc, ident[:])

            def wseg(nm, rows=128):
                off = _W_OFF[nm]
                w = dict(_W_SEGS)[nm]
                return wb[0:rows, off:off + w]

            def b1seg(nm, w):
                off = _B1_OFF[nm]
                return b1[:, off:off + w]

            # persistent result buffers
            vbuf8 = cs.tile([128, TILES * 8], F32)
            ibuf8 = cs.tile([128, TILES * 8], U32)
            wbuf = cs.tile([128, TILES, 2], F32)
            obuf = cs.tile([128, TILES, 2], I32)

            # =========================================================
            # TTHA adapter (fp32, replicated per core)
            # =========================================================
            catT = wseg("catT").rearrange("p (c o) -> p c o", c=3)
            wc = wseg("wc").rearrange("p (c n) -> p c n", c=3)
            ps_emb = ptt.tile([1, 2 * H], F32, tag="ptt")
            for c in range(3):
                nc.tensor.matmul(ps_emb[:, 0:H], catT[:, c, :], wc[:, c, :],
                                 start=(c == 0), stop=(c == 2))
            nc.tensor.matmul(ps_emb[:, H:2 * H], wseg("cat_hw", rows=8),
                             wseg("wh", rows=8), start=True, stop=True)
            e0 = ts.tile([1, 2 * H], F32)
            nc.vector.tensor_tensor(e0[:], ps_emb[:], b1seg("embb", 512), op=OP.add)
            emb_act = ts.tile([1, 2 * H], F32)
            _layer_norm_gelu(nc, ts, e0[:], b1seg("ln1g", 512), b1seg("ln1b", 512),
                             emb_act[:], groups=2, width=H, tag="ln1")

            # combT [128, 2, 3]
            combT = ts.tile([128, 2, 3], F32)
            nc.vector.memset(combT[:], 0.0)
            for c in range(2):
                ps_cb = ptt.tile([128, 2], F32, tag="ptt")
                for j in range(2):
                    nc.tensor.transpose(ps_cb[:, j:j + 1],
                                        emb_act[0:1, (j * H + c * 128):(j * H + c * 128 + 128)],
                                        ident[0:1, 0:1])
                nc.vector.tensor_copy(combT[:, c, 0:2], ps_cb[:])

            # qkv = comb @ Wqkv + bqkv -> [3, 768]
            wqkv = wseg("wqkv").rearrange("p (c n) -> p c n", c=2)
            qkv_s = ts.tile([3, 3 * H], F32)
            for half in range(2):
                ps_q = ptt.tile([3, 384], F32, tag="ptt")
                for c in range(2):
                    nc.tensor.matmul(ps_q[:], combT[:, c, :],
                                     wqkv[:, c, (half * 384):(half * 384 + 384)],
                                     start=(c == 0), stop=(c == 1))
                nc.vector.tensor_tensor(qkv_s[:, half * 384:(half + 1) * 384], ps_q[:],
                                        b3[:, half * 384:(half + 1) * 384], op=OP.add)

            # qT (scaled) / kT packed 3 heads per chunk
            qT = ts.tile([128, 3, 3], F32)
            kT = ts.tile([128, 3, 3], F32)
            scale = 1.0 / np.sqrt(32.0)
            for c in range(3):
                w = 96 if c < 2 else 64
                ps_t = ptt.tile([128, 3], F32, tag="ptt")
                nc.tensor.transpose(ps_t[:w], qkv_s[0:3, c * 96:(c * 96 + w)],
                                    ident[0:3, 0:3])
                nc.vector.tensor_scalar(qT[:w, c, :], ps_t[:w], scale, None, op0=OP.mult)
                ps_t2 = ptt.tile([128, 3], F32, tag="ptt")
                nc.tensor.transpose(ps_t2[:w], qkv_s[0:3, (H + c * 96):(H + c * 96 + w)],
                                    ident[0:3, 0:3])
                nc.vector.tensor_copy(kT[:w, c, :], ps_t2[:w])

            ps_sc = ptt.tile([3, 24], F32, tag="ptt")
            for h in range(8):
                c, hh = divmod(h, 3)
                nc.tensor.matmul(ps_sc[:, h * 3:(h + 1) * 3],
                                 qT[hh * 32:(hh + 1) * 32, c, :],
                                 kT[hh * 32:(hh + 1) * 32, c, :],
                                 start=True, stop=True)
            s_sc = ts.tile([3, 8, 3], F32)
            nc.vector.tensor_copy(s_sc[:], ps_sc[:].rearrange("p (h j) -> p h j", h=8))
            rmax = ts.tile([3, 8], F32)
            nc.vector.tensor_reduce(rmax[:], s_sc[:], axis=AX.X, op=OP.max)
            nc.vector.tensor_tensor(s_sc[:], s_sc[:],
                                    rmax[:, :, None].to_broadcast([3, 8, 3]), op=OP.subtract)
            _act(nc, s_sc[:], s_sc[:], AF.Exp)
            rsum = ts.tile([3, 8], F32)
            nc.vector.tensor_reduce(rsum[:], s_sc[:], axis=AX.X, op=OP.add)
            nc.vector.reciprocal(rsum[:], rsum[:])
            nc.vector.tensor_tensor(s_sc[:], s_sc[:],
                                    rsum[:, :, None].to_broadcast([3, 8, 3]), op=OP.mult)

            at_s = ts.tile([3, 8, 3], F32)
            ps_at = ptt.tile([3, 24], F32, tag="ptt")
            for h in range(8):
                nc.tensor.transpose(ps_at[:, h * 3:(h + 1) * 3], s_sc[:, h, :],
                                    ident[0:3, 0:3])
            nc.vector.tensor_copy(at_s[:], ps_at[:].rearrange("p (h j) -> p h j", h=8))
            ps_ctx = ptt.tile([3, H], F32, tag="ptt")
            for h in range(8):
                nc.tensor.matmul(ps_ctx[:, h * 32:(h + 1) * 32], at_s[:, h, :],
                                 qkv_s[0:3, (2 * H + h * 32):(2 * H + (h + 1) * 32)],
                                 start=True, stop=True)
            ctx_s = ts.tile([3, H], F32)
            nc.vector.tensor_copy(ctx_s[:], ps_ctx[:])

            ctxT = ts.tile([128, 2, 3], F32)
            for c in range(2):
                ps_ct = ptt.tile([128, 3], F32, tag="ptt")
                nc.tensor.transpose(ps_ct[:], ctx_s[0:3, c * 128:(c + 1) * 128],
                                    ident[0:3, 0:3])
                nc.vector.tensor_copy(ctxT[:, c, :], ps_ct[:])
            wo = wseg("wo").rearrange("p (c n) -> p c n", c=2)
            ps_att = ptt.tile([3, H], F32, tag="ptt")
            for c in range(2):
                nc.tensor.matmul(ps_att[:], ctxT[:, c, :], wo[:, c, :],
                                 start=(c == 0), stop=(c == 1))
            att_s = ts.tile([3, H], F32)
            nc.vector.tensor_tensor(att_s[:], ps_att[:], b3[:, 768:768 + H], op=OP.add)

            third = ts.tile([3, 1], F32)
            nc.vector.memset(third[:], 1.0 / 3.0)
            ps_mean = ptt.tile([1, H], F32, tag="ptt")
            nc.tensor.matmul(ps_mean[:], third[:], att_s[:], start=True, stop=True)
            mean_s = ts.tile([1, H], F32)
            nc.vector.tensor_copy(mean_s[:], ps_mean[:])

            meanT = ts.tile([128, 2, 1], F32)
            for c in range(2):
                ps_mt = ptt.tile([128, 1], F32, tag="ptt")
                nc.tensor.transpose(ps_mt[:], mean_s[0:1, c * 128:(c + 1) * 128],
                                    ident[0:1, 0:1])
                nc.vector.tensor_copy(meanT[:, c, :], ps_mt[:])
            wf = wseg("wf").rearrange("p (c n) -> p c n", c=2)
            ps_f = ptt.tile([1, H], F32, tag="ptt")
            for c in range(2):
                nc.tensor.matmul(ps_f[:], meanT[:, c, :], wf[:, c, :],
                                 start=(c == 0), stop=(c == 1))
            f0 = ts.tile([1, H], F32)
            nc.vector.tensor_tensor(f0[:], ps_f[:], b1seg("bf", H), op=OP.add)
            fused = ts.tile([1, H], F32)
            _layer_norm_gelu(nc, ts, f0[:], b1seg("gf", H), b1seg("bef", H), fused[:],
                             groups=1, width=H, tag="ln2")

            fusedT = ts.tile([128, 2, 1], F32)
            for c in range(2):
                ps_ft = ptt.tile([128, 1], F32, tag="ptt")
                nc.tensor.transpose(ps_ft[:], fused[0:1, c * 128:(c + 1) * 128],
                                    ident[0:1, 0:1])
                nc.vector.tensor_copy(fusedT[:, c, :], ps_ft[:])

            # heads: compute both first-layer matmuls, then batch the gelus
            wo1 = wseg("wo1").rearrange("p (c n) -> p c n", c=2)
            ps_h1 = ptt.tile([1, H // 2], F32, tag="ptt")
            for c in range(2):
                nc.tensor.matmul(ps_h1[:], fusedT[:, c, :], wo1[:, c, :],
                                 start=(c == 0), stop=(c == 1))
            wu1 = wseg("wu1").rearrange("p (c n) -> p c n", c=2)
            ps_u1 = ptt.tile([1, H // 4], F32, tag="ptt")
            for c in range(2):
                nc.tensor.matmul(ps_u1[:], fusedT[:, c, :], wu1[:, c, :],
                                 start=(c == 0), stop=(c == 1))
            h1 = ts.tile([1, H // 2], F32)
            nc.vector.tensor_tensor(h1[:], ps_h1[:], b1seg("bo1", H // 2), op=OP.add)
            u1 = ts.tile([1, H // 4], F32)
            nc.vector.tensor_tensor(u1[:], ps_u1[:], b1seg("bu1", H // 4), op=OP.add)
            _act(nc, h1[:], h1[:], AF.Gelu)
            _act(nc, u1[:], u1[:], AF.Gelu)

            h1T = ts.tile([128, 1], F32)
            ps_h1t = ptt.tile([128, 1], F32, tag="ptt")
            nc.tensor.transpose(ps_h1t[:], h1[0:1, :], ident[0:1, 0:1])
            nc.vector.tensor_copy(h1T[:], ps_h1t[:])
            ps_rb = ptt.tile([1, E], F32, tag="ptt")
            nc.tensor.matmul(ps_rb[:], h1T[:], wseg("wo2"), start=True, stop=True)
            rb = ts.tile([1, E], F32)
            nc.vector.tensor_tensor(rb[:], ps_rb[:], b1seg("bo2", E), op=OP.add)
            _act(nc, rb[:], rb[:], AF.Tanh)  # tanh lives in the gelu set
            nc.gpsimd.dma_start(out_rb[:], rb[:])

            u1T = ts.tile([64, 1], F32)
            ps_u1t = ptt.tile([64, 1], F32, tag="ptt")
            nc.tensor.transpose(ps_u1t[:], u1[0:1, :], ident[0:1, 0:1])
            nc.vector.tensor_copy(u1T[:], ps_u1t[:])
            ps_u2 = ptt.tile([1, E], F32, tag="ptt")
            nc.tensor.matmul(ps_u2[:], u1T[:], wseg("wu2", rows=64), start=True, stop=True)
            un = ts.tile([1, E], F32)
            nc.vector.tensor_tensor(un[:], ps_u2[:], b1seg("bu2", E), op=OP.add)
            _act(nc, un[:], un[:], AF.Exp)
            nc.vector.tensor_scalar(un[:], un[:], 1.0, None, op0=OP.add)
            _act(nc, un[:], un[:], AF.Ln)
            nc.gpsimd.dma_start(out_un[:], un[:])

            bt = ts.tile([1, E], F32)
            nc.vector.tensor_tensor(bt[:], rb[:], b1seg("bgate", E), op=OP.add)
            ps_bt = ptt.tile([64, 1], F32, tag="ptt")
            nc.tensor.transpose(ps_bt[:], bt[0:1, :], ident[0:1, 0:1])
            bias_T = ts.tile([64, 1], F32)
            nc.vector.tensor_copy(bias_T[:], ps_bt[:])

            # =========================================================
            # Main gate matmul + top-2
            # =========================================================
            for g in range(N_GROUPS):
                xg = slabs[g]
                ps_lg = pmain.tile([E, G_TOK], F32, tag="lg")
                n_mm = 3 * DC
                k = 0
                for c in range(DC):
                    # (whi, xhi), (whi, xlo), (wlo, xhi)
                    for (wc_i, xc_i) in ((c, c), (c, DC + c), (DC + c, c)):
                        nc.tensor.matmul(ps_lg[:], wz_s[:, wc_i, :], xg[:, xc_i, :],
                                         start=(k == 0), stop=(k == n_mm - 1))
                        k += 1

                # fused bias add on Scalar engine (Identity is in every set)
                adj = wk.tile([E, G_TOK], F32, tag="adj")
                nc.scalar.activation(adj[:], ps_lg[:], AF.Identity,
                                     bias=bias_T[:, 0:1], scale=1.0)

                for t in range(4):
                    tl = g * 4 + t
                    ps_tr = ptr.tile([128, E], F32, tag="tr")
                    nc.tensor.transpose(ps_tr[:], adj[:, t * 128:(t + 1) * 128],
                                        ident[0:E, 0:E])
                    adjT = wk.tile([128, E], F32, tag="adjT")
                    nc.vector.tensor_copy(adjT[:], ps_tr[:])
                    nc.vector.max(vbuf8[:, tl * 8:(tl + 1) * 8], adjT[:])
                    nc.vector.max_index(ibuf8[:, tl * 8:(tl + 1) * 8],
                                        vbuf8[:, tl * 8:(tl + 1) * 8], adjT[:])

            # ---- top-2 softmax epilogue ----
            v3 = vbuf8[:].rearrange("p (t k) -> p t k", k=8)
            i3 = ibuf8[:].rearrange("p (t k) -> p t k", k=8)
            d_t = wk.tile([128, TILES, 1], F32, tag="dt")
            nc.vector.tensor_tensor(d_t[:], v3[:, :, 1:2], v3[:, :, 0:1], op=OP.subtract)
            _act(nc, d_t[:], d_t[:], AF.Exp)
            s_t = wk.tile([128, TILES, 1], F32, tag="st")
            nc.vector.tensor_scalar(s_t[:], d_t[:], 1.0, None, op0=OP.add)
            nc.vector.reciprocal(wbuf[:, :, 0:1], s_t[:])
            nc.vector.tensor_tensor(wbuf[:, :, 1:2], d_t[:], wbuf[:, :, 0:1], op=OP.mult)
            nc.vector.tensor_copy(obuf[:, :, 0:1], i3[:, :, 0:1])
            nc.vector.tensor_copy(obuf[:, :, 1:2], i3[:, :, 1:2])
            nc.sync.dma_start(out_w[:], wbuf[:])
            nc.sync.dma_start(out_i[:], obuf[:])

    nc.finalize()
    return nc


_CACHE = {}


def _get_nc():
    if "nc" not in _CACHE:
        _CACHE["nc"] = build()
    return _CACHE["nc"]


def kernel(**inputs):
    f32 = np.float32
    g = {k: np.asarray(v, f32) for k, v in inputs.items()}
    x = g["x"]

    wg = g["W_gate"]
    wghi = wg.astype(BF)
    wglo = (wg - wghi.astype(f32)).astype(BF)
    wgz = np.concatenate([wghi, wglo], axis=0)

    wblob = np.zeros((128, WCOLS), f32)

    def put_w(nm, arr):
        off = _W_OFF[nm]
        arr = np.asarray(arr, f32)
        wblob[:arr.shape[0], off:off + arr.shape[1]] = arr

    cost = g["cost_features"][0]
    put_w("catT", np.ascontiguousarray(cost.reshape(3, 128).T))
    put_w("cat_hw", g["hardware_features"].reshape(8, 1))
    put_w("wc", g["Wc"].reshape(3, 128, 256).transpose(1, 0, 2).reshape(128, 3 * 256))
    put_w("wh", g["Wh"])
    put_w("wqkv", g["Wqkv"].reshape(2, 128, 768).transpose(1, 0, 2).reshape(128, 2 * 768))
    put_w("wo", g["Wo"].reshape(2, 128, 256).transpose(1, 0, 2).reshape(128, 2 * 256))
    put_w("wf", g["Wf"].reshape(2, 128, 256).transpose(1, 0, 2).reshape(128, 2 * 256))
    put_w("wo1", g["Wo1"].reshape(2, 128, 128).transpose(1, 0, 2).reshape(128, 2 * 128))
    put_w("wo2", g["Wo2"])
    put_w("wu1", g["Wu1"].reshape(2, 128, 64).transpose(1, 0, 2).reshape(128, 2 * 64))
    put_w("wu2", g["Wu2"])

    bblob1 = np.zeros((1, B1COLS), f32)

    def put_b(nm, arr):
        off = _B1_OFF[nm]
        arr = np.asarray(arr, f32).reshape(-1)
        bblob1[0, off:off + arr.size] = arr

    put_b("embb", np.concatenate([g["bc"], g["bh"]]))
    put_b("ln1g", np.concatenate([g["gc"], g["gh"]]))
    put_b("ln1b", np.concatenate([g["bec"], g["beh"]]))
    put_b("bf", g["bf"]); put_b("gf", g["gf"]); put_b("bef", g["bef"])
    put_b("bo1", g["bo1"]); put_b("bo2", g["bo2"])
    put_b("bu1", g["bu1"]); put_b("bu2", g["bu2"])
    put_b("bgate", g["b_gate"])

    bblob3 = np.zeros((3, 1024), f32)
    bblob3[:, 0:768] = g["bqkv"].reshape(1, 768)
    bblob3[:, 768:1024] = g["bo"].reshape(1, 256)

    shared = dict(wgz=wgz, wblob=wblob, bblob1=bblob1, bblob3=bblob3)

    in_maps = []
    for c in range(N_CORES):
        xs = np.ascontiguousarray(x[c * NT:(c + 1) * NT].T)
        xhi = xs.astype(BF)
        xlo = (xs - xhi.astype(f32)).astype(BF)
        in_maps.append(dict(shared, xz=np.concatenate([xhi, xlo], axis=0)))

    nc = _get_nc()
    res = run_bass_kernel_spmd(nc, in_maps, core_ids=list(range(N_CORES)))

    weights = np.empty((N, K), f32)
    top_idx = np.empty((N, K), np.int32)
    for c in range(N_CORES):
        r = res.results[c]
        weights[c * NT:(c + 1) * NT] = (
            r["out_w"].reshape(128, TILES, 2).transpose(1, 0, 2).reshape(NT, 2))
        top_idx[c * NT:(c + 1) * NT] = (
            r["out_i"].reshape(128, TILES, 2).transpose(1, 0, 2).reshape(NT, 2))
    rb = res.results[0]["out_rb"].reshape(1, E).astype(f32)
    un = res.results[0]["out_un"].reshape(1, E).astype(f32)
    return weights, top_idx, rb, un


# revision 15
# speedup vs baseline: 1.1866x; 1.0372x over previous
"""AdaptiveRouter (MoE routing) Trainium2 kernel — 8-core data-parallel.

Strategy:
  - Shard token dim N=32768 across 8 cores (4096 tokens each).
  - Host-side input prep (layout only):
      * each core's x shard is transposed to [D=1024, 4096], split into bf16
        hi/lo halves, and stacked into one [2048, 4096] tensor so the gate
        matmul runs as 3 bf16 matmuls (x_hi@W_hi + x_hi@W_lo + x_lo@W_hi)
        with near-fp32 accuracy (measured ~5e-6 abs err) at bf16 TensorE rate.
      * all small TTHA weights are packed into 3 blob tensors (one
        partition-major [128,*] f32 blob, one [3,*] blob, one [1,*] blob)
        so weight loading is 3 DMA issues instead of ~22.
  - x-slab DMAs are issued FIRST on the sync engine (full 8-group prefetch,
    bufs=8) so the TensorE pipeline starts immediately; weight/TTHA DMAs go
    through gpsimd (SWDGE) to stay off the sync HWDGE FIFO.
  - The tiny batch-1 TTHA adapter is computed on every core (replicated,
    fp32, overlaps the x DMA stream). Activation-table switching minimized:
    Ln/Exp set for layernorm-rsqrt/softmax/softplus, Gelu set (which also
    holds tanh) for the rest; Identity (present in every set) is used for
    the fused bias-add epilogue on the Scalar engine.
  - Per 512-token group: 24 bf16 matmuls -> PSUM [64experts, 512tok],
    ACT Identity adds (b_gate + routing_bias) per partition on the psum->
    sbuf copy, PE-transpose to [128tok, 64], DVE Max8/MaxIndex top-2,
    batched top-2 softmax epilogue.
  - No collectives; host gathers per-core outputs.
"""

import sys

sys.path.insert(0, "/opt/trn_rl_repo")

import numpy as np
import ml_dtypes

import concourse.bass as bass
import concourse.mybir as mybir
import concourse.tile as tile
from concourse import bacc
from concourse.bass_utils import run_bass_kernel_spmd
from concourse.masks import make_identity

F32 = mybir.dt.float32
BF16 = mybir.dt.bfloat16
I32 = mybir.dt.int32
U32 = mybir.dt.uint32
AF = mybir.ActivationFunctionType
OP = mybir.AluOpType
AX = mybir.AxisListType

N_CORES = 8
N, D, E, K = 32768, 1024, 64, 2
NT = N // N_CORES            # 4096 tokens per core
H = 256
G_TOK = 512                  # tokens per matmul group
N_GROUPS = NT // G_TOK       # 8
TILES = NT // 128            # 32 token tiles of 128
DC = D // 128                # 8 contraction chunks

BF = ml_dtypes.bfloat16

# ---- weight blobs (partition-major [128, *] f32) ----
# blob A: needed at the start of the TTHA chain
_WA_SEGS = [("catT", 3), ("cat_hw", 1), ("wc", 3 * 256), ("wh", 256)]
# blob B: needed a few microseconds later
_WB_SEGS = [
    ("wqkv", 2 * 768), ("wo", 2 * 256), ("wf", 2 * 256),
    ("wo1", 2 * 128), ("wo2", 64), ("wu1", 2 * 64), ("wu2", 64),
]


def _offsets(segs):
    out, off = {}, 0
    for nm, w in segs:
        out[nm] = off
        off += w
    return out, off


_WA_OFF, WACOLS = _offsets(_WA_SEGS)
_WB_OFF, WBCOLS = _offsets(_WB_SEGS)

_B1_SEGS = [
    ("embb", 512), ("ln1g", 512), ("ln1b", 512), ("bf", 256), ("gf", 256),
    ("bef", 256), ("bo1", 128), ("bo2", 64), ("bu1", 64), ("bu2", 64),
    ("bgate", 64),
]
_B1_OFF = {}
_off = 0
for _nm, _w in _B1_SEGS:
    _B1_OFF[_nm] = _off
    _off += _w
B1COLS = _off


def _act(nc, out, in_, func, scale=1.0, bias=0.0):
    nc.scalar.activation(out, in_, func, scale=scale, bias=bias)


def _layer_norm_gelu(nc, sb, x_ap, g_ap, b_ap, out_ap, groups, width, tag):
    """out = gelu(LN(x) * g + b) on one partition; LN per group of `width`."""
    x3 = x_ap.rearrange("o (g w) -> o g w", g=groups)
    g3 = g_ap.rearrange("o (g w) -> o g w", g=groups)
    b3 = b_ap.rearrange("o (g w) -> o g w", g=groups)
    o3 = out_ap.rearrange("o (g w) -> o g w", g=groups)

    mu = sb.tile([1, groups], F32, tag=f"{tag}_mu")
    nc.vector.tensor_reduce(mu[:], x3, axis=AX.X, op=OP.add)
    nc.vector.tensor_scalar(mu[:], mu[:], 1.0 / width, None, op0=OP.mult)

    xc = sb.tile([1, groups, width], F32, tag=f"{tag}_xc")
    nc.vector.tensor_tensor(xc[:], x3, mu[:, :, None].to_broadcast([1, groups, width]),
                            op=OP.subtract)
    sq = sb.tile([1, groups, width], F32, tag=f"{tag}_sq")
    nc.vector.tensor_tensor(sq[:], xc[:], xc[:], op=OP.mult)
    var = sb.tile([1, groups], F32, tag=f"{tag}_var")
    nc.vector.tensor_reduce(var[:], sq[:], axis=AX.X, op=OP.add)
    nc.vector.tensor_scalar(var[:], var[:], 1.0 / width, 1e-5, op0=OP.mult, op1=OP.add)
    # inv_std = exp(-0.5 * ln(var+eps))   (stays in the natural_log_exp set)
    _act(nc, var[:], var[:], AF.Ln)
    _act(nc, var[:], var[:], AF.Exp, scale=-0.5)

    nc.vector.tensor_tensor(xc[:], xc[:], var[:, :, None].to_broadcast([1, groups, width]),
                            op=OP.mult)
    nc.vector.tensor_tensor(xc[:], xc[:], g3, op=OP.mult)
    nc.vector.tensor_tensor(xc[:], xc[:], b3, op=OP.add)
    _act(nc, o3, xc[:], AF.Gelu)


def build():
    nc = bacc.Bacc(target_bir_lowering=False)

    xz = nc.dram_tensor("xz", [2 * D, NT], BF16, kind="ExternalInput")
    wgz = nc.dram_tensor("wgz", [2 * D, E], BF16, kind="ExternalInput")
    wbloba = nc.dram_tensor("wbloba", [128, WACOLS], F32, kind="ExternalInput")
    wblobb = nc.dram_tensor("wblobb", [128, WBCOLS], F32, kind="ExternalInput")
    bblob1 = nc.dram_tensor("bblob1", [1, B1COLS], F32, kind="ExternalInput")
    bblob3 = nc.dram_tensor("bblob3", [3, 1024], F32, kind="ExternalInput")

    out_w = nc.dram_tensor("out_w", [128, TILES * 2], F32, kind="ExternalOutput")
    out_i = nc.dram_tensor("out_i", [128, TILES * 2], I32, kind="ExternalOutput")
    out_rb = nc.dram_tensor("out_rb", [1, E], F32, kind="ExternalOutput")
    out_un = nc.dram_tensor("out_un", [1, E], F32, kind="ExternalOutput")

    with tile.TileContext(nc) as tc:
        with tc.tile_pool(name="const", bufs=1) as cs, \
             tc.tile_pool(name="tt", bufs=1) as ts, \
             tc.tile_pool(name="xs", bufs=8) as xs, \
             tc.tile_pool(name="wk", bufs=3) as wk, \
             tc.tile_pool(name="pmain", bufs=4, space="PSUM") as pmain, \
             tc.tile_pool(name="ptr", bufs=2, space="PSUM") as ptr, \
             tc.tile_pool(name="ptt", bufs=2, space="PSUM") as ptt:

            # ---- all input DMAs on the sync engine in hand-picked FIFO order:
            # gate weights -> early TTHA weights -> slab0 -> rest of TTHA
            # weights -> remaining slabs. Slab hi/lo halves are separate 1MB
            # DMAs so the (whi,xhi) products can start before the lo half.
            wz_s = cs.tile([128, 2 * DC, E], BF16)
            nc.sync.dma_start(wz_s[:], wgz[:].rearrange("(c p) e -> p c e", p=128))
            wba = cs.tile([128, WACOLS], F32)
            nc.sync.dma_start(wba[:], wbloba[:])

            def slab_dma(g):
                xhi_g = xs.tile([128, DC, G_TOK], BF16, tag="xhi")
                xlo_g = xs.tile([128, DC, G_TOK], BF16, tag="xlo")
                nc.sync.dma_start(
                    xhi_g[:], xz[0:D, g * G_TOK:(g + 1) * G_TOK].rearrange(
                        "(c p) t -> p c t", p=128))
                nc.sync.dma_start(
                    xlo_g[:], xz[D:2 * D, g * G_TOK:(g + 1) * G_TOK].rearrange(
                        "(c p) t -> p c t", p=128))
                return (xhi_g, xlo_g)

            slabs = [slab_dma(0)]
            wbb = cs.tile([128, WBCOLS], F32)
            nc.sync.dma_start(wbb[:], wblobb[:])
            b1 = cs.tile([1, B1COLS], F32)
            nc.sync.dma_start(b1[:], bblob1[:])
            b3 = cs.tile([3, 1024], F32)
            nc.sync.dma_start(b3[:], bblob3[:])
            for g in range(1, N_GROUPS):
                slabs.append(slab_dma(g))

            ident = cs.tile([128, 128], F32)
            make_identity(nc, ident[:])

            def wseg(nm, rows=128):
                if nm in _WA_OFF:
                    off = _WA_OFF[nm]
                    w = dict(_WA_SEGS)[nm]
                    return wba[0:rows, off:off + w]
                off = _WB_OFF[nm]
                w = dict(_WB_SEGS)[nm]
                return wbb[0:rows, off:off + w]

            def b1seg(nm, w):
                off = _B1_OFF[nm]
                return b1[:, off:off + w]

            # persistent result buffers
            vbuf8 = cs.tile([128, TILES * 8], F32)
            ibuf8 = cs.tile([128, TILES * 8], U32)
            wbuf = cs.tile([128, TILES, 2], F32)
            obuf = cs.tile([128, TILES, 2], I32)

            # =========================================================
            # TTHA adapter (fp32, replicated per core)
            # =========================================================
            _hp = tc.high_priority()
            _hp.__enter__()
            catT = wseg("catT").rearrange("p (c o) -> p c o", c=3)
            wc = wseg("wc").rearrange("p (c n) -> p c n", c=3)
            ps_emb = ptt.tile([1, 2 * H], F32, tag="ptt")
            for c in range(3):
                nc.tensor.matmul(ps_emb[:, 0:H], catT[:, c, :], wc[:, c, :],
                                 start=(c == 0), stop=(c == 2))
            nc.tensor.matmul(ps_emb[:, H:2 * H], wseg("cat_hw", rows=8),
                             wseg("wh", rows=8), start=True, stop=True)
            e0 = ts.tile([1, 2 * H], F32)
            nc.vector.tensor_tensor(e0[:], ps_emb[:], b1seg("embb", 512), op=OP.add)
            emb_act = ts.tile([1, 2 * H], F32)
            _layer_norm_gelu(nc, ts, e0[:], b1seg("ln1g", 512), b1seg("ln1b", 512),
                             emb_act[:], groups=2, width=H, tag="ln1")

            # combT [128, 2, 3]
            combT = ts.tile([128, 2, 3], F32)
            nc.vector.memset(combT[:], 0.0)
            for c in range(2):
                ps_cb = ptt.tile([128, 2], F32, tag="ptt")
                for j in range(2):
                    nc.tensor.transpose(ps_cb[:, j:j + 1],
                                        emb_act[0:1, (j * H + c * 128):(j * H + c * 128 + 128)],
                                        ident[0:1, 0:1])
                nc.vector.tensor_copy(combT[:, c, 0:2], ps_cb[:])

            # qkv = comb @ Wqkv + bqkv -> [3, 768]
            wqkv = wseg("wqkv").rearrange("p (c n) -> p c n", c=2)
            qkv_s = ts.tile([3, 3 * H], F32)
            for half in range(2):
                ps_q = ptt.tile([3, 384], F32, tag="ptt")
                for c in range(2):
                    nc.tensor.matmul(ps_q[:], combT[:, c, :],
                                     wqkv[:, c, (half * 384):(half * 384 + 384)],
                                     start=(c == 0), stop=(c == 1))
                nc.vector.tensor_tensor(qkv_s[:, half * 384:(half + 1) * 384], ps_q[:],
                                        b3[:, half * 384:(half + 1) * 384], op=OP.add)

            # qT (scaled) / kT packed 3 heads per chunk
            qT = ts.tile([128, 3, 3], F32)
            kT = ts.tile([128, 3, 3], F32)
            scale = 1.0 / np.sqrt(32.0)
            for c in range(3):
                w = 96 if c < 2 else 64
                ps_t = ptt.tile([128, 3], F32, tag="ptt")
                nc.tensor.transpose(ps_t[:w], qkv_s[0:3, c * 96:(c * 96 + w)],
                                    ident[0:3, 0:3])
                nc.vector.tensor_scalar(qT[:w, c, :], ps_t[:w], scale, None, op0=OP.mult)
                ps_t2 = ptt.tile([128, 3], F32, tag="ptt")
                nc.tensor.transpose(ps_t2[:w], qkv_s[0:3, (H + c * 96):(H + c * 96 + w)],
                                    ident[0:3, 0:3])
                nc.vector.tensor_copy(kT[:w, c, :], ps_t2[:w])

            ps_sc = ptt.tile([3, 24], F32, tag="ptt")
            for h in range(8):
                c, hh = divmod(h, 3)
                nc.tensor.matmul(ps_sc[:, h * 3:(h + 1) * 3],
                                 qT[hh * 32:(hh + 1) * 32, c, :],
                                 kT[hh * 32:(hh + 1) * 32, c, :],
                                 start=True, stop=True)
            s_sc = ts.tile([3, 8, 3], F32)
            nc.vector.tensor_copy(s_sc[:], ps_sc[:].rearrange("p (h j) -> p h j", h=8))
            rmax = ts.tile([3, 8], F32)
            nc.vector.tensor_reduce(rmax[:], s_sc[:], axis=AX.X, op=OP.max)
            nc.vector.tensor_tensor(s_sc[:], s_sc[:],
                                    rmax[:, :, None].to_broadcast([3, 8, 3]), op=OP.subtract)
            _act(nc, s_sc[:], s_sc[:], AF.Exp)
            rsum = ts.tile([3, 8], F32)
            nc.vector.tensor_reduce(rsum[:], s_sc[:], axis=AX.X, op=OP.add)
            nc.vector.reciprocal(rsum[:], rsum[:])
            nc.vector.tensor_tensor(s_sc[:], s_sc[:],
                                    rsum[:, :, None].to_broadcast([3, 8, 3]), op=OP.mult)

            at_s = ts.tile([3, 8, 3], F32)
            ps_at = ptt.tile([3, 24], F32, tag="ptt")
            for h in range(8):
                nc.tensor.transpose(ps_at[:, h * 3:(h + 1) * 3], s_sc[:, h, :],
                                    ident[0:3, 0:3])
            nc.vector.tensor_copy(at_s[:], ps_at[:].rearrange("p (h j) -> p h j", h=8))
            ps_ctx = ptt.tile([3, H], F32, tag="ptt")
            for h in range(8):
                nc.tensor.matmul(ps_ctx[:, h * 32:(h + 1) * 32], at_s[:, h, :],
                                 qkv_s[0:3, (2 * H + h * 32):(2 * H + (h + 1) * 32)],
                                 start=True, stop=True)
            ctx_s = ts.tile([3, H], F32)
            nc.vector.tensor_copy(ctx_s[:], ps_ctx[:])

            ctxT = ts.tile([128, 2, 3], F32)
            for c in range(2):
                ps_ct = ptt.tile([128, 3], F32, tag="ptt")
                nc.tensor.transpose(ps_ct[:], ctx_s[0:3, c * 128:(c + 1) * 128],
                                    ident[0:3, 0:3])
                nc.vector.tensor_copy(ctxT[:, c, :], ps_ct[:])
            wo = wseg("wo").rearrange("p (c n) -> p c n", c=2)
            ps_att = ptt.tile([3, H], F32, tag="ptt")
            for c in range(2):
                nc.tensor.matmul(ps_att[:], ctxT[:, c, :], wo[:, c, :],
                                 start=(c == 0), stop=(c == 1))
            att_s = ts.tile([3, H], F32)
            nc.vector.tensor_tensor(att_s[:], ps_att[:], b3[:, 768:768 + H], op=OP.add)

            third = ts.tile([3, 1], F32)
            nc.vector.memset(third[:], 1.0 / 3.0)
            ps_mean = ptt.tile([1, H], F32, tag="ptt")
            nc.tensor.matmul(ps_mean[:], third[:], att_s[:], start=True, stop=True)
            mean_s = ts.tile([1, H], F32)
            nc.vector.tensor_copy(mean_s[:], ps_mean[:])

            meanT = ts.tile([128, 2, 1], F32)
            for c in range(2):
                ps_mt = ptt.tile([128, 1], F32, tag="ptt")
                nc.tensor.transpose(ps_mt[:], mean_s[0:1, c * 128:(c + 1) * 128],
                                    ident[0:1, 0:1])
                nc.vector.tensor_copy(meanT[:, c, :], ps_mt[:])
            wf = wseg("wf").rearrange("p (c n) -> p c n", c=2)
            ps_f = ptt.tile([1, H], F32, tag="ptt")
            for c in range(2):
                nc.tensor.matmul(ps_f[:], meanT[:, c, :], wf[:, c, :],
                                 start=(c == 0), stop=(c == 1))
            f0 = ts.tile([1, H], F32)
            nc.vector.tensor_tensor(f0[:], ps_f[:], b1seg("bf", H), op=OP.add)
            fused = ts.tile([1, H], F32)
            _layer_norm_gelu(nc, ts, f0[:], b1seg("gf", H), b1seg("bef", H), fused[:],
                             groups=1, width=H, tag="ln2")

            fusedT = ts.tile([128, 2, 1], F32)
            for c in range(2):
                ps_ft = ptt.tile([128, 1], F32, tag="ptt")
                nc.tensor.transpose(ps_ft[:], fused[0:1, c * 128:(c + 1) * 128],
                                    ident[0:1, 0:1])
                nc.vector.tensor_copy(fusedT[:, c, :], ps_ft[:])

            # heads: compute both first-layer matmuls, then batch the gelus
            wo1 = wseg("wo1").rearrange("p (c n) -> p c n", c=2)
            ps_h1 = ptt.tile([1, H // 2], F32, tag="ptt")
            for c in range(2):
                nc.tensor.matmul(ps_h1[:], fusedT[:, c, :], wo1[:, c, :],
                                 start=(c == 0), stop=(c == 1))
            wu1 = wseg("wu1").rearrange("p (c n) -> p c n", c=2)
            ps_u1 = ptt.tile([1, H // 4], F32, tag="ptt")
            for c in range(2):
                nc.tensor.matmul(ps_u1[:], fusedT[:, c, :], wu1[:, c, :],
                                 start=(c == 0), stop=(c == 1))
            h1 = ts.tile([1, H // 2], F32)
            nc.vector.tensor_tensor(h1[:], ps_h1[:], b1seg("bo1", H // 2), op=OP.add)
            u1 = ts.tile([1, H // 4], F32)
            nc.vector.tensor_tensor(u1[:], ps_u1[:], b1seg("bu1", H // 4), op=OP.add)
            _act(nc, h1[:], h1[:], AF.Gelu)
            _act(nc, u1[:], u1[:], AF.Gelu)

            h1T = ts.tile([128, 1], F32)
            ps_h1t = ptt.tile([128, 1], F32, tag="ptt")
            nc.tensor.transpose(ps_h1t[:], h1[0:1, :], ident[0:1, 0:1])
            nc.vector.tensor_copy(h1T[:], ps_h1t[:])
            ps_rb = ptt.tile([1, E], F32, tag="ptt")
            nc.tensor.matmul(ps_rb[:], h1T[:], wseg("wo2"), start=True, stop=True)
            rb = ts.tile([1, E], F32)
            nc.vector.tensor_tensor(rb[:], ps_rb[:], b1seg("bo2", E), op=OP.add)
            _act(nc, rb[:], rb[:], AF.Tanh)  # tanh lives in the gelu set
            nc.gpsimd.dma_start(out_rb[:], rb[:])

            u1T = ts.tile([64, 1], F32)
            ps_u1t = ptt.tile([64, 1], F32, tag="ptt")
            nc.tensor.transpose(ps_u1t[:], u1[0:1, :], ident[0:1, 0:1])
            nc.vector.tensor_copy(u1T[:], ps_u1t[:])
            ps_u2 = ptt.tile([1, E], F32, tag="ptt")
            nc.tensor.matmul(ps_u2[:], u1T[:], wseg("wu2", rows=64), start=True, stop=True)
            un = ts.tile([1, E], F32)
            nc.vector.tensor_tensor(un[:], ps_u2[:], b1seg("bu2", E), op=OP.add)
            _act(nc, un[:], un[:], AF.Exp)
            nc.vector.tensor_scalar(un[:], un[:], 1.0, None, op0=OP.add)
            _act(nc, un[:], un[:], AF.Ln)
            nc.gpsimd.dma_start(out_un[:], un[:])

            bt = ts.tile([1, E], F32)
            nc.vector.tensor_tensor(bt[:], rb[:], b1seg("bgate", E), op=OP.add)
            ps_bt = ptt.tile([64, 1], F32, tag="ptt")
            nc.tensor.transpose(ps_bt[:], bt[0:1, :], ident[0:1, 0:1])
            bias_T = ts.tile([64, 1], F32)
            nc.vector.tensor_copy(bias_T[:], ps_bt[:])
            _hp.__exit__(None, None, None)

            # =========================================================
            # Main gate matmul + top-2
            # =========================================================
            for g in range(N_GROUPS):
                xhi_g, xlo_g = slabs[g]
                ps_lg = pmain.tile([E, G_TOK], F32, tag="lg")
                n_mm = 3 * DC
                k = 0
                # hi products first (only need the hi half of the slab)
                for c in range(DC):
                    nc.tensor.matmul(ps_lg[:], wz_s[:, c, :], xhi_g[:, c, :],
                                     start=(k == 0), stop=False)
                    k += 1
                for c in range(DC):
                    nc.tensor.matmul(ps_lg[:], wz_s[:, c, :], xlo_g[:, c, :],
                                     start=False, stop=False)
                    k += 1
                    nc.tensor.matmul(ps_lg[:], wz_s[:, DC + c, :], xhi_g[:, c, :],
                                     start=False, stop=(k == n_mm - 1))
                    k += 1

                # fused bias add on Scalar engine (Identity is in every set)
                adj = wk.tile([E, G_TOK], F32, tag="adj")
                nc.scalar.activation(adj[:], ps_lg[:], AF.Identity,
                                     bias=bias_T[:, 0:1], scale=1.0)

                for t in range(4):
                    tl = g * 4 + t
                    ps_tr = ptr.tile([128, E], F32, tag="tr")
                    nc.tensor.transpose(ps_tr[:], adj[:, t * 128:(t + 1) * 128],
                                        ident[0:E, 0:E])
                    adjT = wk.tile([128, E], F32, tag="adjT")
                    nc.vector.tensor_copy(adjT[:], ps_tr[:])
                    nc.vector.max(vbuf8[:, tl * 8:(tl + 1) * 8], adjT[:])
                    nc.vector.max_index(ibuf8[:, tl * 8:(tl + 1) * 8],
                                        vbuf8[:, tl * 8:(tl + 1) * 8], adjT[:])

            # ---- top-2 softmax epilogue ----
            v3 = vbuf8[:].rearrange("p (t k) -> p t k", k=8)
            i3 = ibuf8[:].rearrange("p (t k) -> p t k", k=8)
            d_t = wk.tile([128, TILES, 1], F32, tag="dt")
            nc.vector.tensor_tensor(d_t[:], v3[:, :, 1:2], v3[:, :, 0:1], op=OP.subtract)
            _act(nc, d_t[:], d_t[:], AF.Exp)
            s_t = wk.tile([128, TILES, 1], F32, tag="st")
            nc.vector.tensor_scalar(s_t[:], d_t[:], 1.0, None, op0=OP.add)
            nc.vector.reciprocal(wbuf[:, :, 0:1], s_t[:])
            nc.vector.tensor_tensor(wbuf[:, :, 1:2], d_t[:], wbuf[:, :, 0:1], op=OP.mult)
            nc.vector.tensor_copy(obuf[:, :, 0:1], i3[:, :, 0:1])
            nc.vector.tensor_copy(obuf[:, :, 1:2], i3[:, :, 1:2])
            nc.sync.dma_start(out_w[:], wbuf[:])
            nc.sync.dma_start(out_i[:], obuf[:])

    nc.finalize()
    return nc


_CACHE = {}


def _get_nc():
    if "nc" not in _CACHE:
        _CACHE["nc"] = build()
    return _CACHE["nc"]


def kernel(**inputs):
    f32 = np.float32
    g = {k: np.asarray(v, f32) for k, v in inputs.items()}
    x = g["x"]

    wg = g["W_gate"]
    wghi = wg.astype(BF)
    wglo = (wg - wghi.astype(f32)).astype(BF)
    wgz = np.concatenate([wghi, wglo], axis=0)

    wbloba = np.zeros((128, WACOLS), f32)
    wblobb = np.zeros((128, WBCOLS), f32)

    def put_w(nm, arr):
        blob, off = ((wbloba, _WA_OFF[nm]) if nm in _WA_OFF
                     else (wblobb, _WB_OFF[nm]))
        arr = np.asarray(arr, f32)
        blob[:arr.shape[0], off:off + arr.shape[1]] = arr

    cost = g["cost_features"][0]
    put_w("catT", np.ascontiguousarray(cost.reshape(3, 128).T))
    put_w("cat_hw", g["hardware_features"].reshape(8, 1))
    put_w("wc", g["Wc"].reshape(3, 128, 256).transpose(1, 0, 2).reshape(128, 3 * 256))
    put_w("wh", g["Wh"])
    put_w("wqkv", g["Wqkv"].reshape(2, 128, 768).transpose(1, 0, 2).reshape(128, 2 * 768))
    put_w("wo", g["Wo"].reshape(2, 128, 256).transpose(1, 0, 2).reshape(128, 2 * 256))
    put_w("wf", g["Wf"].reshape(2, 128, 256).transpose(1, 0, 2).reshape(128, 2 * 256))
    put_w("wo1", g["Wo1"].reshape(2, 128, 128).transpose(1, 0, 2).reshape(128, 2 * 128))
    put_w("wo2", g["Wo2"])
    put_w("wu1", g["Wu1"].reshape(2, 128, 64).transpose(1, 0, 2).reshape(128, 2 * 64))
    put_w("wu2", g["Wu2"])

    bblob1 = np.zeros((1, B1COLS), f32)

    def put_b(nm, arr):
        off = _B1_OFF[nm]
        arr = np.asarray(arr, f32).reshape(-1)
        bblob1[0, off:off + arr.size] = arr

    put_b("embb", np.concatenate([g["bc"], g["bh"]]))
    put_b("ln1g", np.concatenate([g["gc"], g["gh"]]))
    put_b("ln1b", np.concatenate([g["bec"], g["beh"]]))
    put_b("bf", g["bf"]); put_b("gf", g["gf"]); put_b("bef", g["bef"])
    put_b("bo1", g["bo1"]); put_b("bo2", g["bo2"])
    put_b("bu1", g["bu1"]); put_b("bu2", g["bu2"])
    put_b("bgate", g["b_gate"])

    bblob3 = np.zeros((3, 1024), f32)
    bblob3[:, 0:768] = g["bqkv"].reshape(1, 768)
    bblob3[:, 768:1024] = g["bo"].reshape(1, 256)

    shared = dict(wgz=wgz, wbloba=wbloba, wblobb=wblobb, bblob1=bblob1,
                  bblob3=bblob3)

    in_maps = []
    for c in range(N_CORES):
        xs = np.ascontiguousarray(x[c * NT:(c + 1) * NT].T)
        xhi = xs.astype(BF)
        xlo = (xs - xhi.astype(f32)).astype(BF)
        in_maps.append(dict(shared, xz=np.concatenate([xhi, xlo], axis=0)))

    nc = _get_nc()
    res = run_bass_kernel_spmd(nc, in_maps, core_ids=list(range(N_CORES)))

    weights = np.empty((N, K), f32)
    top_idx = np.empty((N, K), np.int32)
    for c in range(N_CORES):
        r = res.results[c]
        weights[c * NT:(c + 1) * NT] = (
            r["out_w"].reshape(128, TILES, 2).transpose(1, 0, 2).reshape(NT, 2))
        top_idx[c * NT:(c + 1) * NT] = (
            r["out_i"].reshape(128, TILES, 2).transpose(1, 0, 2).reshape(NT, 2))
    rb = res.results[0]["out_rb"].reshape(1, E).astype(f32)
    un = res.results[0]["out_un"].reshape(1, E).astype(f32)
    return weights, top_idx, rb, un


# revision 17
# speedup vs baseline: 1.2738x; 1.0735x over previous
"""AdaptiveRouter (MoE routing) Trainium2 kernel — 8-core data-parallel.

Strategy:
  - Shard token dim N=32768 across 8 cores (4096 tokens each).
  - Host-side input prep (layout only):
      * each core's x shard is transposed to [D=1024, 4096], split into bf16
        hi/lo halves, and stacked into one [2048, 4096] tensor so the gate
        matmul runs as 3 bf16 matmuls (x_hi@W_hi + x_hi@W_lo + x_lo@W_hi)
        with near-fp32 accuracy (measured ~5e-6 abs err) at bf16 TensorE rate.
      * all small TTHA weights are packed into 3 blob tensors (one
        partition-major [128,*] f32 blob, one [3,*] blob, one [1,*] blob)
        so weight loading is 3 DMA issues instead of ~22.
  - x-slab DMAs are issued FIRST on the sync engine (full 8-group prefetch,
    bufs=8) so the TensorE pipeline starts immediately; weight/TTHA DMAs go
    through gpsimd (SWDGE) to stay off the sync HWDGE FIFO.
  - The tiny batch-1 TTHA adapter is computed on every core (replicated,
    fp32, overlaps the x DMA stream). Activation-table switching minimized:
    Ln/Exp set for layernorm-rsqrt/softmax/softplus, Gelu set (which also
    holds tanh) for the rest; Identity (present in every set) is used for
    the fused bias-add epilogue on the Scalar engine.
  - Per 512-token group: 24 bf16 matmuls -> PSUM [64experts, 512tok],
    ACT Identity adds (b_gate + routing_bias) per partition on the psum->
    sbuf copy, PE-transpose to [128tok, 64], DVE Max8/MaxIndex top-2,
    batched top-2 softmax epilogue.
  - No collectives; host gathers per-core outputs.
"""

import sys

sys.path.insert(0, "/opt/trn_rl_repo")

import numpy as np
import ml_dtypes

import concourse.bass as bass
import concourse.mybir as mybir
import concourse.tile as tile
from concourse import bacc
from concourse.bass_utils import run_bass_kernel_spmd
from concourse.masks import make_identity

F32 = mybir.dt.float32
BF16 = mybir.dt.bfloat16
I32 = mybir.dt.int32
U32 = mybir.dt.uint32
AF = mybir.ActivationFunctionType
OP = mybir.AluOpType
AX = mybir.AxisListType

N_CORES = 8
N, D, E, K = 32768, 1024, 64, 2
NT = N // N_CORES            # 4096 tokens per core
H = 256
G_TOK = 512                  # tokens per matmul group
N_GROUPS = NT // G_TOK       # 8
TILES = NT // 128            # 32 token tiles of 128
DC = D // 128                # 8 contraction chunks

BF = ml_dtypes.bfloat16

# ---- weight blobs (partition-major [128, *] f32) ----
# blob A: needed at the start of the TTHA chain
_WA_SEGS = [("catT", 3), ("cat_hw", 1), ("wc", 3 * 256), ("wh", 256)]
# blob B: needed a few microseconds later
_WB_SEGS = [
    ("wqkv", 2 * 768), ("wo", 2 * 256), ("wf", 2 * 256),
    ("wo1", 2 * 128), ("wo2", 64), ("wu1", 2 * 64), ("wu2", 64),
]


def _offsets(segs):
    out, off = {}, 0
    for nm, w in segs:
        out[nm] = off
        off += w
    return out, off


_WA_OFF, WACOLS = _offsets(_WA_SEGS)
_WB_OFF, WBCOLS = _offsets(_WB_SEGS)

_B1_SEGS = [
    ("embb", 512), ("ln1g", 512), ("ln1b", 512), ("bf", 256), ("gf", 256),
    ("bef", 256), ("bo1", 128), ("bo2", 64), ("bu1", 64), ("bu2", 64),
    ("bgate", 64),
]
_B1_OFF = {}
_off = 0
for _nm, _w in _B1_SEGS:
    _B1_OFF[_nm] = _off
    _off += _w
B1COLS = _off


def _act(nc, out, in_, func, scale=1.0, bias=0.0):
    nc.scalar.activation(out, in_, func, scale=scale, bias=bias)


def _layer_norm_gelu(nc, sb, x_ap, g_ap, b_ap, out_ap, groups, width, tag):
    """out = gelu(LN(x) * g + b) on one partition; LN per group of `width`."""
    x3 = x_ap.rearrange("o (g w) -> o g w", g=groups)
    g3 = g_ap.rearrange("o (g w) -> o g w", g=groups)
    b3 = b_ap.rearrange("o (g w) -> o g w", g=groups)
    o3 = out_ap.rearrange("o (g w) -> o g w", g=groups)

    mu = sb.tile([1, groups], F32, tag=f"{tag}_mu")
    nc.vector.tensor_reduce(mu[:], x3, axis=AX.X, op=OP.add)
    nc.vector.tensor_scalar(mu[:], mu[:], 1.0 / width, None, op0=OP.mult)

    xc = sb.tile([1, groups, width], F32, tag=f"{tag}_xc")
    nc.vector.tensor_tensor(xc[:], x3, mu[:, :, None].to_broadcast([1, groups, width]),
                            op=OP.subtract)
    sq = sb.tile([1, groups, width], F32, tag=f"{tag}_sq")
    nc.vector.tensor_tensor(sq[:], xc[:], xc[:], op=OP.mult)
    var = sb.tile([1, groups], F32, tag=f"{tag}_var")
    nc.vector.tensor_reduce(var[:], sq[:], axis=AX.X, op=OP.add)
    nc.vector.tensor_scalar(var[:], var[:], 1.0 / width, 1e-5, op0=OP.mult, op1=OP.add)
    # inv_std = exp(-0.5 * ln(var+eps))   (stays in the natural_log_exp set)
    _act(nc, var[:], var[:], AF.Ln)
    _act(nc, var[:], var[:], AF.Exp, scale=-0.5)

    nc.vector.tensor_tensor(xc[:], xc[:], var[:, :, None].to_broadcast([1, groups, width]),
                            op=OP.mult)
    nc.vector.tensor_tensor(xc[:], xc[:], g3, op=OP.mult)
    nc.vector.tensor_tensor(xc[:], xc[:], b3, op=OP.add)
    _act(nc, o3, xc[:], AF.Gelu)


def build():
    nc = bacc.Bacc(target_bir_lowering=False)

    xz = nc.dram_tensor("xz", [2 * D, NT], BF16, kind="ExternalInput")
    wgz = nc.dram_tensor("wgz", [2 * D, E], BF16, kind="ExternalInput")
    wbloba = nc.dram_tensor("wbloba", [128, WACOLS], F32, kind="ExternalInput")
    wblobb = nc.dram_tensor("wblobb", [128, WBCOLS], F32, kind="ExternalInput")
    bblob1 = nc.dram_tensor("bblob1", [1, B1COLS], F32, kind="ExternalInput")
    bblob3 = nc.dram_tensor("bblob3", [3, 1024], F32, kind="ExternalInput")

    out_w = nc.dram_tensor("out_w", [128, TILES * 2], F32, kind="ExternalOutput")
    out_i = nc.dram_tensor("out_i", [128, TILES * 2], I32, kind="ExternalOutput")
    out_rb = nc.dram_tensor("out_rb", [1, E], F32, kind="ExternalOutput")
    out_un = nc.dram_tensor("out_un", [1, E], F32, kind="ExternalOutput")

    with tile.TileContext(nc) as tc:
        with tc.tile_pool(name="const", bufs=1) as cs, \
             tc.tile_pool(name="tt", bufs=1) as ts, \
             tc.tile_pool(name="xs", bufs=8) as xs, \
             tc.tile_pool(name="wk", bufs=3) as wk, \
             tc.tile_pool(name="pmain", bufs=4, space="PSUM") as pmain, \
             tc.tile_pool(name="ptr", bufs=2, space="PSUM") as ptr, \
             tc.tile_pool(name="ptt", bufs=2, space="PSUM") as ptt:

            # ---- DMA plan: the sync engine carries ONLY the 16 x-slab halves
            # (the bandwidth-critical stream); all weights/biases ride the
            # gpsimd SWDGE queue, with the early-TTHA blob first.
            wba = cs.tile([128, WACOLS], F32)
            nc.gpsimd.dma_start(wba[:], wbloba[:])
            wz_s = cs.tile([128, 2 * DC, E], BF16)
            nc.gpsimd.dma_start(wz_s[:], wgz[:].rearrange("(c p) e -> p c e", p=128))
            wbb = cs.tile([128, WBCOLS], F32)
            nc.gpsimd.dma_start(wbb[:], wblobb[:])
            b1 = cs.tile([1, B1COLS], F32)
            nc.gpsimd.dma_start(b1[:], bblob1[:])
            b3 = cs.tile([3, 1024], F32)
            nc.gpsimd.dma_start(b3[:], bblob3[:])

            slabs = []
            for g in range(N_GROUPS):
                xhi_g = xs.tile([128, DC, G_TOK], BF16, tag="xhi")
                xlo_g = xs.tile([128, DC, G_TOK], BF16, tag="xlo")
                nc.sync.dma_start(
                    xhi_g[:], xz[0:D, g * G_TOK:(g + 1) * G_TOK].rearrange(
                        "(c p) t -> p c t", p=128))
                nc.sync.dma_start(
                    xlo_g[:], xz[D:2 * D, g * G_TOK:(g + 1) * G_TOK].rearrange(
                        "(c p) t -> p c t", p=128))
                slabs.append((xhi_g, xlo_g))

            ident = cs.tile([128, 128], F32)
            make_identity(nc, ident[:])

            def wseg(nm, rows=128):
                if nm in _WA_OFF:
                    off = _WA_OFF[nm]
                    w = dict(_WA_SEGS)[nm]
                    return wba[0:rows, off:off + w]
                off = _WB_OFF[nm]
                w = dict(_WB_SEGS)[nm]
                return wbb[0:rows, off:off + w]

            def b1seg(nm, w):
                off = _B1_OFF[nm]
                return b1[:, off:off + w]

            # persistent result buffers
            vbuf8 = cs.tile([128, TILES * 8], F32)
            ibuf8 = cs.tile([128, TILES * 8], U32)
            wbuf = cs.tile([128, TILES, 2], F32)
            obuf = cs.tile([128, TILES, 2], I32)

            # =========================================================
            # TTHA adapter (fp32, replicated per core)
            # =========================================================
            _hp = tc.high_priority()
            _hp.__enter__()
            catT = wseg("catT").rearrange("p (c o) -> p c o", c=3)
            wc = wseg("wc").rearrange("p (c n) -> p c n", c=3)
            ps_emb = ptt.tile([1, 2 * H], F32, tag="ptt")
            for c in range(3):
                nc.tensor.matmul(ps_emb[:, 0:H], catT[:, c, :], wc[:, c, :],
                                 start=(c == 0), stop=(c == 2))
            nc.tensor.matmul(ps_emb[:, H:2 * H], wseg("cat_hw", rows=8),
                             wseg("wh", rows=8), start=True, stop=True)
            e0 = ts.tile([1, 2 * H], F32)
            nc.vector.tensor_tensor(e0[:], ps_emb[:], b1seg("embb", 512), op=OP.add)
            emb_act = ts.tile([1, 2 * H], F32)
            _layer_norm_gelu(nc, ts, e0[:], b1seg("ln1g", 512), b1seg("ln1b", 512),
                             emb_act[:], groups=2, width=H, tag="ln1")

            # combT [128, 2, 3]
            combT = ts.tile([128, 2, 3], F32)
            nc.vector.memset(combT[:], 0.0)
            for c in range(2):
                ps_cb = ptt.tile([128, 2], F32, tag="ptt")
                for j in range(2):
                    nc.tensor.transpose(ps_cb[:, j:j + 1],
                                        emb_act[0:1, (j * H + c * 128):(j * H + c * 128 + 128)],
                                        ident[0:1, 0:1])
                nc.vector.tensor_copy(combT[:, c, 0:2], ps_cb[:])

            # qkv = comb @ Wqkv + bqkv -> [3, 768]
            wqkv = wseg("wqkv").rearrange("p (c n) -> p c n", c=2)
            qkv_s = ts.tile([3, 3 * H], F32)
            for half in range(2):
                ps_q = ptt.tile([3, 384], F32, tag="ptt")
                for c in range(2):
                    nc.tensor.matmul(ps_q[:], combT[:, c, :],
                                     wqkv[:, c, (half * 384):(half * 384 + 384)],
                                     start=(c == 0), stop=(c == 1))
                nc.vector.tensor_tensor(qkv_s[:, half * 384:(half + 1) * 384], ps_q[:],
                                        b3[:, half * 384:(half + 1) * 384], op=OP.add)

            # qT (scaled) / kT packed 3 heads per chunk
            qT = ts.tile([128, 3, 3], F32)
            kT = ts.tile([128, 3, 3], F32)
            scale = 1.0 / np.sqrt(32.0)
            for c in range(3):
                w = 96 if c < 2 else 64
                ps_t = ptt.tile([128, 3], F32, tag="ptt")
                nc.tensor.transpose(ps_t[:w], qkv_s[0:3, c * 96:(c * 96 + w)],
                                    ident[0:3, 0:3])
                nc.vector.tensor_scalar(qT[:w, c, :], ps_t[:w], scale, None, op0=OP.mult)
                ps_t2 = ptt.tile([128, 3], F32, tag="ptt")
                nc.tensor.transpose(ps_t2[:w], qkv_s[0:3, (H + c * 96):(H + c * 96 + w)],
                                    ident[0:3, 0:3])
                nc.vector.tensor_copy(kT[:w, c, :], ps_t2[:w])

            ps_sc = ptt.tile([3, 24], F32, tag="ptt")
            for h in range(8):
                c, hh = divmod(h, 3)
                nc.tensor.matmul(ps_sc[:, h * 3:(h + 1) * 3],
                                 qT[hh * 32:(hh + 1) * 32, c, :],
                                 kT[hh * 32:(hh + 1) * 32, c, :],
                                 start=True, stop=True)
            s_sc = ts.tile([3, 8, 3], F32)
            nc.vector.tensor_copy(s_sc[:], ps_sc[:].rearrange("p (h j) -> p h j", h=8))
            rmax = ts.tile([3, 8], F32)
            nc.vector.tensor_reduce(rmax[:], s_sc[:], axis=AX.X, op=OP.max)
            nc.vector.tensor_tensor(s_sc[:], s_sc[:],
                                    rmax[:, :, None].to_broadcast([3, 8, 3]), op=OP.subtract)
            _act(nc, s_sc[:], s_sc[:], AF.Exp)
            rsum = ts.tile([3, 8], F32)
            nc.vector.tensor_reduce(rsum[:], s_sc[:], axis=AX.X, op=OP.add)
            nc.vector.reciprocal(rsum[:], rsum[:])
            nc.vector.tensor_tensor(s_sc[:], s_sc[:],
                                    rsum[:, :, None].to_broadcast([3, 8, 3]), op=OP.mult)

            at_s = ts.tile([3, 8, 3], F32)
            ps_at = ptt.tile([3, 24], F32, tag="ptt")
            for h in range(8):
                nc.tensor.transpose(ps_at[:, h * 3:(h + 1) * 3], s_sc[:, h, :],
                                    ident[0:3, 0:3])
            nc.vector.tensor_copy(at_s[:], ps_at[:].rearrange("p (h j) -> p h j", h=8))
            ps_ctx = ptt.tile([3, H], F32, tag="ptt")
            for h in range(8):
                nc.tensor.matmul(ps_ctx[:, h * 32:(h + 1) * 32], at_s[:, h, :],
                                 qkv_s[0:3, (2 * H + h * 32):(2 * H + (h + 1) * 32)],
                                 start=True, stop=True)
            ctx_s = ts.tile([3, H], F32)
            nc.vector.tensor_copy(ctx_s[:], ps_ctx[:])

            ctxT = ts.tile([128, 2, 3], F32)
            for c in range(2):
                ps_ct = ptt.tile([128, 3], F32, tag="ptt")
                nc.tensor.transpose(ps_ct[:], ctx_s[0:3, c * 128:(c + 1) * 128],
                                    ident[0:3, 0:3])
                nc.vector.tensor_copy(ctxT[:, c, :], ps_ct[:])
            wo = wseg("wo").rearrange("p (c n) -> p c n", c=2)
            ps_att = ptt.tile([3, H], F32, tag="ptt")
            for c in range(2):
                nc.tensor.matmul(ps_att[:], ctxT[:, c, :], wo[:, c, :],
                                 start=(c == 0), stop=(c == 1))
            att_s = ts.tile([3, H], F32)
            nc.vector.tensor_tensor(att_s[:], ps_att[:], b3[:, 768:768 + H], op=OP.add)

            third = ts.tile([3, 1], F32)
            nc.vector.memset(third[:], 1.0 / 3.0)
            ps_mean = ptt.tile([1, H], F32, tag="ptt")
            nc.tensor.matmul(ps_mean[:], third[:], att_s[:], start=True, stop=True)
            mean_s = ts.tile([1, H], F32)
            nc.vector.tensor_copy(mean_s[:], ps_mean[:])

            meanT = ts.tile([128, 2, 1], F32)
            for c in range(2):
                ps_mt = ptt.tile([128, 1], F32, tag="ptt")
                nc.tensor.transpose(ps_mt[:], mean_s[0:1, c * 128:(c + 1) * 128],
                                    ident[0:1, 0:1])
                nc.vector.tensor_copy(meanT[:, c, :], ps_mt[:])
            wf = wseg("wf").rearrange("p (c n) -> p c n", c=2)
            ps_f = ptt.tile([1, H], F32, tag="ptt")
            for c in range(2):
                nc.tensor.matmul(ps_f[:], meanT[:, c, :], wf[:, c, :],
                                 start=(c == 0), stop=(c == 1))
            f0 = ts.tile([1, H], F32)
            nc.vector.tensor_tensor(f0[:], ps_f[:], b1seg("bf", H), op=OP.add)
            fused = ts.tile([1, H], F32)
            _layer_norm_gelu(nc, ts, f0[:], b1seg("gf", H), b1seg("bef", H), fused[:],
                             groups=1, width=H, tag="ln2")

            fusedT = ts.tile([128, 2, 1], F32)
            for c in range(2):
                ps_ft = ptt.tile([128, 1], F32, tag="ptt")
                nc.tensor.transpose(ps_ft[:], fused[0:1, c * 128:(c + 1) * 128],
                                    ident[0:1, 0:1])
                nc.vector.tensor_copy(fusedT[:, c, :], ps_ft[:])

            # heads: compute both first-layer matmuls, then batch the gelus
            wo1 = wseg("wo1").rearrange("p (c n) -> p c n", c=2)
            ps_h1 = ptt.tile([1, H // 2], F32, tag="ptt")
            for c in range(2):
                nc.tensor.matmul(ps_h1[:], fusedT[:, c, :], wo1[:, c, :],
                                 start=(c == 0), stop=(c == 1))
            wu1 = wseg("wu1").rearrange("p (c n) -> p c n", c=2)
            ps_u1 = ptt.tile([1, H // 4], F32, tag="ptt")
            for c in range(2):
                nc.tensor.matmul(ps_u1[:], fusedT[:, c, :], wu1[:, c, :],
                                 start=(c == 0), stop=(c == 1))
            h1 = ts.tile([1, H // 2], F32)
            nc.vector.tensor_tensor(h1[:], ps_h1[:], b1seg("bo1", H // 2), op=OP.add)
            u1 = ts.tile([1, H // 4], F32)
            nc.vector.tensor_tensor(u1[:], ps_u1[:], b1seg("bu1", H // 4), op=OP.add)
            _act(nc, h1[:], h1[:], AF.Gelu)
            _act(nc, u1[:], u1[:], AF.Gelu)

            h1T = ts.tile([128, 1], F32)
            ps_h1t = ptt.tile([128, 1], F32, tag="ptt")
            nc.tensor.transpose(ps_h1t[:], h1[0:1, :], ident[0:1, 0:1])
            nc.vector.tensor_copy(h1T[:], ps_h1t[:])
            ps_rb = ptt.tile([1, E], F32, tag="ptt")
            nc.tensor.matmul(ps_rb[:], h1T[:], wseg("wo2"), start=True, stop=True)
            rb = ts.tile([1, E], F32)
            nc.vector.tensor_tensor(rb[:], ps_rb[:], b1seg("bo2", E), op=OP.add)
            _act(nc, rb[:], rb[:], AF.Tanh)  # tanh lives in the gelu set
            nc.gpsimd.dma_start(out_rb[:], rb[:])

            u1T = ts.tile([64, 1], F32)
            ps_u1t = ptt.tile([64, 1], F32, tag="ptt")
            nc.tensor.transpose(ps_u1t[:], u1[0:1, :], ident[0:1, 0:1])
            nc.vector.tensor_copy(u1T[:], ps_u1t[:])
            ps_u2 = ptt.tile([1, E], F32, tag="ptt")
            nc.tensor.matmul(ps_u2[:], u1T[:], wseg("wu2", rows=64), start=True, stop=True)
            un = ts.tile([1, E], F32)
            nc.vector.tensor_tensor(un[:], ps_u2[:], b1seg("bu2", E), op=OP.add)
            _act(nc, un[:], un[:], AF.Exp)
            nc.vector.tensor_scalar(un[:], un[:], 1.0, None, op0=OP.add)
            _act(nc, un[:], un[:], AF.Ln)
            nc.gpsimd.dma_start(out_un[:], un[:])

            # total per-expert bias broadcast to all 128 partitions via PE:
            # bias128 = ones[1,128].T @ (rb + b_gate)[1,64]
            bt = ts.tile([1, E], F32)
            nc.vector.tensor_tensor(bt[:], rb[:], b1seg("bgate", E), op=OP.add)
            ones1 = ts.tile([1, 128], F32)
            nc.vector.memset(ones1[:], 1.0)
            ps_b128 = ptt.tile([128, E], F32, tag="ptt")
            nc.tensor.matmul(ps_b128[:], ones1[:], bt[:], start=True, stop=True)
            bias128 = cs.tile([128, E], F32)
            nc.vector.tensor_copy(bias128[:], ps_b128[:])
            _hp.__exit__(None, None, None)

            # =========================================================
            # Main gate matmul + top-2.  The PE pipeline (matmuls +
            # transposes) never waits on the TTHA chain: raw logits are
            # transposed into lbuf, and the bias add + top-2 for group g
            # run one group behind (by which time bias128 is ready).
            # =========================================================
            lbuf = cs.tile([128, TILES, E], F32)

            def gated_top2(g):
                tmp4 = wk.tile([128, 4, E], F32, tag="tmp4")
                nc.vector.tensor_tensor(
                    tmp4[:], lbuf[:, g * 4:(g + 1) * 4, :],
                    bias128[:, None, :].to_broadcast([128, 4, E]), op=OP.add)
                for t in range(4):
                    tl = g * 4 + t
                    nc.vector.max(vbuf8[:, tl * 8:(tl + 1) * 8], tmp4[:, t, :])
                    nc.vector.max_index(ibuf8[:, tl * 8:(tl + 1) * 8],
                                        vbuf8[:, tl * 8:(tl + 1) * 8], tmp4[:, t, :])

            for g in range(N_GROUPS):
                xhi_g, xlo_g = slabs[g]
                ps_lg = pmain.tile([E, G_TOK], F32, tag="lg")
                n_mm = 3 * DC
                k = 0
                # hi products first (only need the hi half of the slab)
                for c in range(DC):
                    nc.tensor.matmul(ps_lg[:], wz_s[:, c, :], xhi_g[:, c, :],
                                     start=(k == 0), stop=False)
                    k += 1
                for c in range(DC):
                    nc.tensor.matmul(ps_lg[:], wz_s[:, c, :], xlo_g[:, c, :],
                                     start=False, stop=False)
                    k += 1
                    nc.tensor.matmul(ps_lg[:], wz_s[:, DC + c, :], xhi_g[:, c, :],
                                     start=False, stop=(k == n_mm - 1))
                    k += 1

                # raw (unbiased) logits to SBUF on the Scalar engine
                adj = wk.tile([E, G_TOK], F32, tag="adj")
                nc.scalar.activation(adj[:], ps_lg[:], AF.Copy)

                for t in range(4):
                    tl = g * 4 + t
                    ps_tr = ptr.tile([128, E], F32, tag="tr")
                    nc.tensor.transpose(ps_tr[:], adj[:, t * 128:(t + 1) * 128],
                                        ident[0:E, 0:E])
                    nc.vector.tensor_copy(lbuf[:, tl, :], ps_tr[:])

                if g >= 1:
                    gated_top2(g - 1)
            gated_top2(N_GROUPS - 1)

            # ---- top-2 softmax epilogue ----
            v3 = vbuf8[:].rearrange("p (t k) -> p t k", k=8)
            i3 = ibuf8[:].rearrange("p (t k) -> p t k", k=8)
            d_t = wk.tile([128, TILES, 1], F32, tag="dt")
            nc.vector.tensor_tensor(d_t[:], v3[:, :, 1:2], v3[:, :, 0:1], op=OP.subtract)
            _act(nc, d_t[:], d_t[:], AF.Exp)
            s_t = wk.tile([128, TILES, 1], F32, tag="st")
            nc.vector.tensor_scalar(s_t[:], d_t[:], 1.0, None, op0=OP.add)
            nc.vector.reciprocal(wbuf[:, :, 0:1], s_t[:])
            nc.vector.tensor_tensor(wbuf[:, :, 1:2], d_t[:], wbuf[:, :, 0:1], op=OP.mult)
            nc.vector.tensor_copy(obuf[:, :, 0:1], i3[:, :, 0:1])
            nc.vector.tensor_copy(obuf[:, :, 1:2], i3[:, :, 1:2])
            nc.sync.dma_start(out_w[:], wbuf[:])
            nc.sync.dma_start(out_i[:], obuf[:])

    nc.finalize()
    return nc


_CACHE = {}


def _get_nc():
    if "nc" not in _CACHE:
        _CACHE["nc"] = build()
    return _CACHE["nc"]


def kernel(**inputs):
    f32 = np.float32
    g = {k: np.asarray(v, f32) for k, v in inputs.items()}
    x = g["x"]

    wg = g["W_gate"]
    wghi = wg.astype(BF)
    wglo = (wg - wghi.astype(f32)).astype(BF)
    wgz = np.concatenate([wghi, wglo], axis=0)

    wbloba = np.zeros((128, WACOLS), f32)
    wblobb = np.zeros((128, WBCOLS), f32)

    def put_w(nm, arr):
        blob, off = ((wbloba, _WA_OFF[nm]) if nm in _WA_OFF
                     else (wblobb, _WB_OFF[nm]))
        arr = np.asarray(arr, f32)
        blob[:arr.shape[0], off:off + arr.shape[1]] = arr

    cost = g["cost_features"][0]
    put_w("catT", np.ascontiguousarray(cost.reshape(3, 128).T))
    put_w("cat_hw", g["hardware_features"].reshape(8, 1))
    put_w("wc", g["Wc"].reshape(3, 128, 256).transpose(1, 0, 2).reshape(128, 3 * 256))
    put_w("wh", g["Wh"])
    put_w("wqkv", g["Wqkv"].reshape(2, 128, 768).transpose(1, 0, 2).reshape(128, 2 * 768))
    put_w("wo", g["Wo"].reshape(2, 128, 256).transpose(1, 0, 2).reshape(128, 2 * 256))
    put_w("wf", g["Wf"].reshape(2, 128, 256).transpose(1, 0, 2).reshape(128, 2 * 256))
    put_w("wo1", g["Wo1"].reshape(2, 128, 128).transpose(1, 0, 2).reshape(128, 2 * 128))
    put_w("wo2", g["Wo2"])
    put_w("wu1", g["Wu1"].reshape(2, 128, 64).transpose(1, 0, 2).reshape(128, 2 * 64))
    put_w("wu2", g["Wu2"])

    bblob1 = np.zeros((1, B1COLS), f32)

    def put_b(nm, arr):
        off = _B1_OFF[nm]
        arr = np.asarray(arr, f32).reshape(-1)
        bblob1[0, off:off + arr.size] = arr

    put_b("embb", np.concatenate([g["bc"], g["bh"]]))
    put_b("ln1g", np.concatenate([g["gc"], g["gh"]]))
    put_b("ln1b", np.concatenate([g["bec"], g["beh"]]))
    put_b("bf", g["bf"]); put_b("gf", g["gf"]); put_b("bef", g["bef"])
    put_b("bo1", g["bo1"]); put_b("bo2", g["bo2"])
    put_b("bu1", g["bu1"]); put_b("bu2", g["bu2"])
    put_b("bgate", g["b_gate"])

    bblob3 = np.zeros((3, 1024), f32)
    bblob3[:, 0:768] = g["bqkv"].reshape(1, 768)
    bblob3[:, 768:1024] = g["bo"].reshape(1, 256)

    shared = dict(wgz=wgz, wbloba=wbloba, wblobb=wblobb, bblob1=bblob1,
                  bblob3=bblob3)

    in_maps = []
    for c in range(N_CORES):
        xs = np.ascontiguousarray(x[c * NT:(c + 1) * NT].T)
        xhi = xs.astype(BF)
        xlo = (xs - xhi.astype(f32)).astype(BF)
        in_maps.append(dict(shared, xz=np.concatenate([xhi, xlo], axis=0)))

    nc = _get_nc()
    res = run_bass_kernel_spmd(nc, in_maps, core_ids=list(range(N_CORES)))

    weights = np.empty((N, K), f32)
    top_idx = np.empty((N, K), np.int32)
    for c in range(N_CORES):
        r = res.results[c]
        weights[c * NT:(c + 1) * NT] = (
            r["out_w"].reshape(128, TILES, 2).transpose(1, 0, 2).reshape(NT, 2))
        top_idx[c * NT:(c + 1) * NT] = (
            r["out_i"].reshape(128, TILES, 2).transpose(1, 0, 2).reshape(NT, 2))
    rb = res.results[0]["out_rb"].reshape(1, E).astype(f32)
    un = res.results[0]["out_un"].reshape(1, E).astype(f32)
    return weights, top_idx, rb, un
